# revision 4
# baseline (speedup 1.0000x reference)
"""Causal self-attention (GQA + RoPE) Trainium2 kernel over 8 NeuronCores.

Sharding: 8 cores = batch(2) x kv_head(4). Each core computes its batch's
4 q-heads / 1 kv-head attention plus the partial output projection; host
sums the 4 partial projections per batch element.

Device algorithm (fully transposed "k-major" attention, zero P-transposes):
  stage A: QKV^T = [wq|wk|wv]^T @ x^T on PE (fp32r)
  RoPE on DVE via interleaved-pair layout (host permutes wq/wk columns so
    rotation partners are adjacent partitions -> stream_shuffle swap)
  V^T -> V via PE transposes; ones-column appended -> PV matmul emits both
    Y^T and softmax sums in one accumulation
  S^T = K^T.T @ Q^T per (tk-tile, tq-chunk); exp on ACT (no max subtraction,
    scores bounded); causal staircase masks multiplied on DVE
  normalize: reciprocal of sums + gpsimd partition_broadcast + DVE mul
  projection: wproj rows for this core's heads, fp32r; partial out to HBM
"""

import sys

sys.path.insert(0, "/opt/trn_rl_repo")

import numpy as np
import ml_dtypes

import concourse.bacc as bacc
import concourse.tile as tile
from concourse import mybir
from concourse.bass_utils import run_bass_kernel_spmd

F32 = mybir.dt.float32
F32R = mybir.dt.float32r
BF16 = mybir.dt.bfloat16
AF = mybir.ActivationFunctionType

T, C, D, H, HKV = 2048, 1024, 64, 16, 4
G = H // HKV  # q heads per kv head
NCC = C // 128  # 8 contraction chunks
NJQ = 4  # tq chunks of 512
TQC = 512
NTK = T // 128  # 16 tk tiles
SCALE = 1.0 / 8.0  # 1/sqrt(D)

_PROG = {}


def _build_program():
    nc = bacc.Bacc()
    xT_d = nc.dram_tensor("xT", [C, T], F32R, kind="ExternalInput")
    w_d = nc.dram_tensor("w_all", [C, 384], F32R, kind="ExternalInput")
    wp_d = nc.dram_tensor("wp", [256, C], F32R, kind="ExternalInput")
    cq_d = nc.dram_tensor("cos_q", [128, T], F32, kind="ExternalInput")
    sq_d = nc.dram_tensor("sin_q", [128, T], F32, kind="ExternalInput")
    mk_d = nc.dram_tensor("masks", [128, 4, TQC], BF16, kind="ExternalInput")
    id_d = nc.dram_tensor("identb", [128, 128], F32, kind="ExternalInput")
    out_d = nc.dram_tensor("out_p", [T, C], F32, kind="ExternalOutput")

    swap_mask = [i ^ 1 for i in range(32)]

    with tile.TileContext(nc) as tc:
        with (
            tc.tile_pool(name="const", bufs=1) as const,
            tc.tile_pool(name="big", bufs=1) as big,
        ):
            # ---- constants ----
            W_sb = const.tile([128, NCC, 384], F32R, tag="W", name="W_sb")
            nc.sync.dma_start(out=W_sb[:], in_=w_d[:].rearrange("(n p) m -> p n m", p=128))
            wp_sb = const.tile([128, 2, C], F32R, tag="wp", name="wp_sb")
            nc.sync.dma_start(out=wp_sb[:], in_=wp_d[:].rearrange("(n p) m -> p n m", p=128))
            cq_sb = const.tile([128, T], F32, tag="cq", name="cq_sb")
            nc.sync.dma_start(out=cq_sb[:], in_=cq_d[:])
            sq_sb = const.tile([128, T], F32, tag="sq", name="sq_sb")
            nc.sync.dma_start(out=sq_sb[:], in_=sq_d[:])
            mk_sb = const.tile([128, 4, TQC], BF16, tag="mk", name="mk_sb")
            nc.sync.dma_start(out=mk_sb[:], in_=mk_d[:])
            id_sb = const.tile([128, 128], F32, tag="idb", name="id_sb")
            nc.sync.dma_start(out=id_sb[:], in_=id_d[:])

            qkv_sb = [big.tile([128, T], F32, tag=f"qkv{m}", name=f"qkv{m}") for m in range(3)]
            qrope = [big.tile([128, T], BF16, tag=f"qr{m}", name=f"qr{m}") for m in range(2)]
            k2 = big.tile([128, T], BF16, tag="k2", name="k2")
            vhat = big.tile([128, NTK, 65], BF16, tag="vhat", name="vhat")
            yn = [big.tile([128, T], F32R, tag=f"yn{m}", name=f"yn{m}") for m in range(2)]

            # ---- stage A: QKV^T = W^T @ x^T (fp32r) + V transposes ----
            with (
                tc.tile_pool(name="xp", bufs=1) as xp,
                tc.tile_pool(name="rope", bufs=1) as ropep,
                tc.tile_pool(name="psA", bufs=2, space="PSUM") as psA,
                tc.tile_pool(name="psT", bufs=2, space="PSUM") as psT,
            ):
                xts = []
                for cc in range(NCC):
                    xt = xp.tile([128, T], F32R, tag=f"x{cc}", name=f"x{cc}")
                    nc.sync.dma_start(out=xt[:], in_=xT_d[cc * 128 : (cc + 1) * 128, :])
                    xts.append(xt)

                for mt in range(3):
                    for jq in range(NJQ):
                        pa = psA.tile([128, TQC], F32, tag="pa", name="pa")
                        for cc in range(NCC):
                            nc.tensor.matmul(
                                pa[:],
                                lhsT=W_sb[:, cc, mt * 128 : (mt + 1) * 128],
                                rhs=xts[cc][:, jq * TQC : (jq + 1) * TQC],
                                start=(cc == 0),
                                stop=(cc == NCC - 1),
                            )
                        nc.scalar.copy(
                            out=qkv_sb[mt][:, jq * TQC : (jq + 1) * TQC], in_=pa[:]
                        )

                # RoPE: interleaved-pair rotation, sign/scale folded into tables
                for pt in range(3):
                    rows = 128 if pt < 2 else 64
                    ct = cq_sb
                    st = sq_sb
                    dst = qrope[pt] if pt < 2 else k2
                    src = qkv_sb[pt]
                    shuf = ropep.tile([128, T], F32, tag="shuf", name="shuf")
                    prod = ropep.tile([128, T], F32, tag="prod", name="prod")
                    nc.vector.stream_shuffle(shuf[:rows, :], src[:rows, :], mask=swap_mask)
                    nc.vector.tensor_mul(out=shuf[:rows, :], in0=shuf[:rows, :], in1=st[:rows, :])
                    nc.vector.tensor_mul(out=prod[:rows, :], in0=src[:rows, :], in1=ct[:rows, :])
                    nc.vector.tensor_add(out=dst[:rows, :], in0=prod[:rows, :], in1=shuf[:rows, :])

                # duplicate K^T into partitions 64:128 (pairs heads for PE row groups)
                nc.sync.dma_start(out=k2[64:128, :], in_=k2[0:64, :])

                # Vhat: V (t-major) + ones column for softmax sums
                nc.vector.memset(vhat[:, :, 64:65], 1.0)
                for tt in range(NTK):
                    pt_ = psT.tile([128, 64], F32, tag="ptr", name="ptr")
                    nc.tensor.transpose(
                        pt_[:],
                        qkv_sb[2][64:128, tt * 128 : (tt + 1) * 128],
                        id_sb[64:128, 0:64],
                    )
                    nc.vector.tensor_copy(out=vhat[:, tt, 0:64], in_=pt_[:])

            # ---- attention + projection ----
            with (
                tc.tile_pool(name="ptiles", bufs=6) as ppool,
                tc.tile_pool(name="small", bufs=3) as small,
                tc.tile_pool(name="outp", bufs=3) as outp,
                tc.tile_pool(name="psS", bufs=2, space="PSUM") as psS,
                tc.tile_pool(name="psY", bufs=4, space="PSUM") as psY,
                tc.tile_pool(name="psP", bufs=2, space="PSUM") as psP,
            ):
                for jq in range(NJQ):
                    nik = 4 * jq + 4
                    pys = [psY.tile([65, TQC], F32, tag="py", name="py") for _ in range(4)]
                    for ik in range(nik):
                        for h in range(4):
                            qt = qrope[h // 2]
                            base = (h % 2) * 64
                            ps_s = psS.tile([128, TQC], F32, tag="ps_s", name="ps_s")
                            nc.tensor.matmul(
                                ps_s[:],
                                lhsT=k2[base : base + 64, ik * 128 : (ik + 1) * 128],
                                rhs=qt[base : base + 64, jq * TQC : (jq + 1) * TQC],
                                start=True,
                                stop=True,
                            )
                            ptile = ppool.tile([128, TQC], BF16, tag="pt", name="ptile")
                            nc.scalar.activation(out=ptile[:], in_=ps_s[:], func=AF.Exp, scale=SCALE)
                            s = ik - 4 * jq
                            if s >= 0:
                                nc.vector.tensor_mul(
                                    out=ptile[:], in0=ptile[:], in1=mk_sb[:, s, :]
                                )
                            nc.tensor.matmul(
                                pys[h][:],
                                lhsT=vhat[:, ik, :],
                                rhs=ptile[:],
                                start=(ik == 0),
                                stop=(ik == nik - 1),
                            )
                    for h in range(4):
                        srow = small.tile([1, TQC], F32, tag="srow", name="srow")
                        nc.vector.tensor_copy(out=srow[:], in_=pys[h][64:65, :])
                        rinv = small.tile([1, TQC], F32, tag="rinv", name="rinv")
                        nc.vector.reciprocal(out=rinv[:], in_=srow[:])
                        rb = small.tile([64, TQC], F32, tag="rb", name="rb")
                        nc.gpsimd.partition_broadcast(rb[:], rinv[:])
                        ybase = (h % 2) * 64
                        nc.vector.tensor_mul(
                            out=yn[h // 2][ybase : ybase + 64, jq * TQC : (jq + 1) * TQC],
                            in0=pys[h][0:64, :],
                            in1=rb[:],
                        )
                    # projection for the 4 t-tiles of this chunk
                    for tt in range(4 * jq, 4 * jq + 4):
                        outsb = outp.tile([128, C], F32, tag="osb", name="osb")
                        for ncol in range(2):
                            pp = psP.tile([128, 512], F32, tag="pp", name="pp")
                            for kk in range(2):
                                nc.tensor.matmul(
                                    pp[:],
                                    lhsT=yn[kk][:, tt * 128 : (tt + 1) * 128],
                                    rhs=wp_sb[:, kk, ncol * 512 : (ncol + 1) * 512],
                                    start=(kk == 0),
                                    stop=(kk == 1),
                                )
                            nc.vector.tensor_copy(
                                out=outsb[:, ncol * 512 : (ncol + 1) * 512], in_=pp[:]
                            )
                        nc.sync.dma_start(
                            out=out_d[tt * 128 : (tt + 1) * 128, :], in_=outsb[:]
                        )

    nc.compile()
    return nc


def _host_tables():
    # RoPE tables in interleaved-pair device layout (row j'=2i <-> orig j=i,
    # j'=2i+1 <-> orig j=i+32); sign of the shuffled sin term folded in.
    inv = 1.0 / (10000.0 ** (np.arange(0, D, 2, dtype=np.float64) / D))  # (32,)
    t = np.arange(T, dtype=np.float64)
    fr = np.outer(t, inv)  # (T, 32)
    cos_h = np.cos(fr).T.astype(np.float32)  # (32, T)
    sin_h = np.sin(fr).T.astype(np.float32)
    cosI = np.empty((D, T), np.float32)
    sinI = np.empty((D, T), np.float32)
    cosI[0::2] = cos_h
    cosI[1::2] = cos_h
    sinI[0::2] = -sin_h
    sinI[1::2] = sin_h
    cos_q = np.tile(cosI, (2, 1))
    sin_q = np.tile(sinI, (2, 1))
    # masks: staircase tile s of a tq-chunk: allowed iff s*128 + tkl <= tql
    tkl = np.arange(128)[:, None]
    tql = np.arange(TQC)[None, :]
    masks = np.stack(
        [(s * 128 + tkl <= tql).astype(np.float32) for s in range(4)], axis=1
    ).astype(ml_dtypes.bfloat16)  # (128, 4, TQC)
    identb = np.tile(np.eye(64, dtype=np.float32), (2, 2))
    return cos_q, sin_q, masks, identb


def make_in_maps(x, wq, wk, wv, wproj):
    cos_q, sin_q, masks, identb = _host_tables()
    # interleave permutation within each head's 64 cols: perm[2i]=i, perm[2i+1]=i+32
    perm = np.empty(D, np.int64)
    perm[0::2] = np.arange(32)
    perm[1::2] = np.arange(32) + 32
    in_maps = []
    for c in range(8):
        b, h = c // 4, c % 4
        xT = np.ascontiguousarray(x[b].T)  # (C, T)
        wq_h = wq[:, h * 256 : (h + 1) * 256].reshape(C, G, D)[:, :, perm].reshape(C, 256)
        wk_h = wk[:, h * 64 : (h + 1) * 64][:, perm]
        wv_h = wv[:, h * 64 : (h + 1) * 64]
        w_all = np.ascontiguousarray(np.concatenate([wq_h, wk_h, wv_h], axis=1))
        wp_h = np.ascontiguousarray(wproj[h * 256 : (h + 1) * 256, :])
        in_maps.append(
            {
                "xT": xT,
                "w_all": w_all,
                "wp": wp_h,
                "cos_q": cos_q,
                "sin_q": sin_q,
                "masks": masks,
                "identb": identb,
            }
        )
    return in_maps


def kernel(x, wq, wk, wv, wproj):
    x = np.asarray(x, dtype=np.float32)
    wq = np.asarray(wq, dtype=np.float32)
    wk = np.asarray(wk, dtype=np.float32)
    wv = np.asarray(wv, dtype=np.float32)
    wproj = np.asarray(wproj, dtype=np.float32)
    B = x.shape[0]

    if "nc" not in _PROG:
        _PROG["nc"] = _build_program()
    nc = _PROG["nc"]

    in_maps = make_in_maps(x, wq, wk, wv, wproj)

    res = run_bass_kernel_spmd(nc, in_maps, list(range(8)))
    out = np.zeros((B, T, C), np.float32)
    for c in range(8):
        out[c // 4] += res.results[c]["out_p"]
    return out


# revision 7
# speedup vs baseline: 1.0987x; 1.0987x over previous
"""Causal self-attention (GQA + RoPE) Trainium2 kernel over 8 NeuronCores.

Sharding: 8 cores = batch(2) x kv_head(4). Each core computes its batch's
4 q-heads / 1 kv-head attention plus the partial output projection; host
sums the 4 partial projections per batch element.

Device algorithm (fully transposed "k-major" attention, zero P-transposes):
  stage A: QKV^T = [wq|wk|wv]^T @ x^T on PE (fp32r)
  RoPE on DVE via interleaved-pair layout (host permutes wq/wk columns so
    rotation partners are adjacent partitions -> stream_shuffle swap)
  V^T -> V via PE transposes; ones-column appended -> PV matmul emits both
    Y^T and softmax sums in one accumulation
  S^T = K^T.T @ Q^T per (tk-tile, tq-chunk); exp on ACT (no max subtraction,
    scores bounded); causal staircase masks multiplied on DVE
  normalize: reciprocal of sums + gpsimd partition_broadcast + DVE mul
  projection: wproj rows for this core's heads, fp32r; partial out to HBM
"""

import sys

sys.path.insert(0, "/opt/trn_rl_repo")

import numpy as np
import ml_dtypes

import concourse.bacc as bacc
import concourse.tile as tile
from concourse import mybir
from concourse.bass_utils import run_bass_kernel_spmd

F32 = mybir.dt.float32
F32R = mybir.dt.float32r
BF16 = mybir.dt.bfloat16
AF = mybir.ActivationFunctionType

T, C, D, H, HKV = 2048, 1024, 64, 16, 4
G = H // HKV  # q heads per kv head
NCC = C // 128  # 8 contraction chunks
NJQ = 4  # tq chunks of 512
TQC = 512
NTK = T // 128  # 16 tk tiles
SCALE = 1.0 / 8.0  # 1/sqrt(D)

_PROG = {}


def _build_program():
    nc = bacc.Bacc()
    xT_d = nc.dram_tensor("xT", [C, T], F32R, kind="ExternalInput")
    w_d = nc.dram_tensor("w_all", [C, 384], F32R, kind="ExternalInput")
    wp_d = nc.dram_tensor("wp", [256, C], F32R, kind="ExternalInput")
    cq_d = nc.dram_tensor("cos_q", [128, T], F32, kind="ExternalInput")
    sq_d = nc.dram_tensor("sin_q", [128, T], F32, kind="ExternalInput")
    mk_d = nc.dram_tensor("masks", [128, 4, TQC], BF16, kind="ExternalInput")
    id_d = nc.dram_tensor("identb", [128, 128], F32, kind="ExternalInput")
    out_d = nc.dram_tensor("out_p", [T, C], F32, kind="ExternalOutput")

    swap_mask = [i ^ 1 for i in range(32)]

    with tile.TileContext(nc) as tc:
        with (
            tc.tile_pool(name="const", bufs=1) as const,
            tc.tile_pool(name="big", bufs=1) as big,
        ):
            # ---- constants (W first so stage A starts ASAP; wp last) ----
            W_sb = const.tile([128, NCC, 384], F32R, tag="W", name="W_sb")
            nc.sync.dma_start(out=W_sb[:], in_=w_d[:].rearrange("(n p) m -> p n m", p=128))
            cq_sb = const.tile([128, T], F32, tag="cq", name="cq_sb")
            sq_sb = const.tile([128, T], F32, tag="sq", name="sq_sb")
            mk_sb = const.tile([128, 4, TQC], BF16, tag="mk", name="mk_sb")
            id_sb = const.tile([128, 128], F32, tag="idb", name="id_sb")
            wp_sb = const.tile([128, 2, C], F32R, tag="wp", name="wp_sb")

            qkv_sb = [big.tile([128, T], F32, tag=f"qkv{m}", name=f"qkv{m}") for m in range(3)]
            qrope = [big.tile([128, T], BF16, tag=f"qr{m}", name=f"qr{m}") for m in range(2)]
            k2 = big.tile([128, T], BF16, tag="k2", name="k2")
            vhat = big.tile([128, NTK, 65], BF16, tag="vhat", name="vhat")
            yn = [big.tile([128, T], F32R, tag=f"yn{m}", name=f"yn{m}") for m in range(2)]

            # ---- stage A: QKV^T = W^T @ x^T (fp32r) + V transposes ----
            with (
                tc.tile_pool(name="xp", bufs=1) as xp,
                tc.tile_pool(name="rope", bufs=1) as ropep,
                tc.tile_pool(name="psA", bufs=2, space="PSUM") as psA,
                tc.tile_pool(name="psT", bufs=2, space="PSUM") as psT,
            ):
                xts = []
                for cc in range(NCC):
                    xt = xp.tile([128, T], F32R, tag=f"x{cc}", name=f"x{cc}")
                    nc.sync.dma_start(out=xt[:], in_=xT_d[cc * 128 : (cc + 1) * 128, :])
                    xts.append(xt)
                nc.sync.dma_start(out=cq_sb[:], in_=cq_d[:])
                nc.sync.dma_start(out=sq_sb[:], in_=sq_d[:])
                nc.sync.dma_start(out=mk_sb[:], in_=mk_d[:])
                nc.sync.dma_start(out=id_sb[:], in_=id_d[:])
                nc.sync.dma_start(out=wp_sb[:], in_=wp_d[:].rearrange("(n p) m -> p n m", p=128))

                for mt in (2, 0, 1):
                    for jq in range(NJQ):
                        pa = psA.tile([128, TQC], F32, tag="pa", name="pa")
                        for cc in range(NCC):
                            nc.tensor.matmul(
                                pa[:],
                                lhsT=W_sb[:, cc, mt * 128 : (mt + 1) * 128],
                                rhs=xts[cc][:, jq * TQC : (jq + 1) * TQC],
                                start=(cc == 0),
                                stop=(cc == NCC - 1),
                            )
                        nc.scalar.copy(
                            out=qkv_sb[mt][:, jq * TQC : (jq + 1) * TQC], in_=pa[:]
                        )

                # RoPE: interleaved-pair rotation, sign/scale folded into tables
                for pt in (2, 0, 1):
                    rows = 128 if pt < 2 else 64
                    ct = cq_sb
                    st = sq_sb
                    dst = qrope[pt] if pt < 2 else k2
                    src = qkv_sb[pt]
                    shuf = ropep.tile([128, T], F32, tag="shuf", name="shuf")
                    prod = ropep.tile([128, T], F32, tag="prod", name="prod")
                    nc.vector.stream_shuffle(shuf[:rows, :], src[:rows, :], mask=swap_mask)
                    nc.vector.tensor_mul(out=shuf[:rows, :], in0=shuf[:rows, :], in1=st[:rows, :])
                    nc.vector.tensor_mul(out=prod[:rows, :], in0=src[:rows, :], in1=ct[:rows, :])
                    nc.vector.tensor_add(out=dst[:rows, :], in0=prod[:rows, :], in1=shuf[:rows, :])

                # duplicate K^T into partitions 64:128 (pairs heads for PE row groups)
                nc.sync.dma_start(out=k2[64:128, :], in_=k2[0:64, :])

                # Vhat: V (t-major) + ones column for softmax sums
                nc.vector.memset(vhat[:, :, 64:65], 1.0)
                for tt in range(NTK):
                    pt_ = psT.tile([128, 64], F32, tag="ptr", name="ptr")
                    nc.tensor.transpose(
                        pt_[:],
                        qkv_sb[2][64:128, tt * 128 : (tt + 1) * 128],
                        id_sb[64:128, 0:64],
                    )
                    nc.vector.tensor_copy(out=vhat[:, tt, 0:64], in_=pt_[:])

            # ---- attention + projection ----
            with (
                tc.tile_pool(name="ptiles", bufs=6) as ppool,
                tc.tile_pool(name="small", bufs=3) as small,
                tc.tile_pool(name="outp", bufs=3) as outp,
                tc.tile_pool(name="psS", bufs=2, space="PSUM") as psS,
                tc.tile_pool(name="psY", bufs=4, space="PSUM") as psY,
                tc.tile_pool(name="psP", bufs=2, space="PSUM") as psP,
            ):

                def emit_proj(pjq):
                    for tt in range(4 * pjq, 4 * pjq + 4):
                        outsb = outp.tile([128, C], F32, tag="osb", name="osb")
                        for ncol in range(2):
                            pp = psP.tile([128, 512], F32, tag="pp", name="pp")
                            for kk in range(2):
                                nc.tensor.matmul(
                                    pp[:],
                                    lhsT=yn[kk][:, tt * 128 : (tt + 1) * 128],
                                    rhs=wp_sb[:, kk, ncol * 512 : (ncol + 1) * 512],
                                    start=(kk == 0),
                                    stop=(kk == 1),
                                )
                            nc.vector.tensor_copy(
                                out=outsb[:, ncol * 512 : (ncol + 1) * 512], in_=pp[:]
                            )
                        nc.sync.dma_start(
                            out=out_d[tt * 128 : (tt + 1) * 128, :], in_=outsb[:]
                        )

                for jq in range(NJQ):
                    nik = 4 * jq + 4
                    pys = [psY.tile([65, TQC], F32, tag="py", name="py") for _ in range(4)]
                    for ik in range(nik):
                        for h in range(4):
                            qt = qrope[h // 2]
                            base = (h % 2) * 64
                            ps_s = psS.tile([128, TQC], F32, tag="ps_s", name="ps_s")
                            nc.tensor.matmul(
                                ps_s[:],
                                lhsT=k2[base : base + 64, ik * 128 : (ik + 1) * 128],
                                rhs=qt[base : base + 64, jq * TQC : (jq + 1) * TQC],
                                start=True,
                                stop=True,
                            )
                            ptile = ppool.tile([128, TQC], BF16, tag="pt", name="ptile")
                            nc.scalar.activation(out=ptile[:], in_=ps_s[:], func=AF.Exp, scale=SCALE)
                            s = ik - 4 * jq
                            if s >= 0:
                                nc.vector.tensor_mul(
                                    out=ptile[:], in0=ptile[:], in1=mk_sb[:, s, :]
                                )
                            nc.tensor.matmul(
                                pys[h][:],
                                lhsT=vhat[:, ik, :],
                                rhs=ptile[:],
                                start=(ik == 0),
                                stop=(ik == nik - 1),
                            )
                    # projection for the PREVIOUS chunk's t-tiles: deferred one
                    # chunk so the in-order PE stream never stalls on the
                    # normalize chain (its yn inputs finished long ago).
                    if jq > 0:
                        emit_proj(jq - 1)
                    for h in range(4):
                        srow = small.tile([1, TQC], F32, tag="srow", name="srow")
                        nc.vector.tensor_copy(out=srow[:], in_=pys[h][64:65, :])
                        rinv = small.tile([1, TQC], F32, tag="rinv", name="rinv")
                        nc.vector.reciprocal_approx_fast(out=rinv[:], in_=srow[:])
                        rb = small.tile([64, TQC], F32, tag="rb", name="rb")
                        nc.gpsimd.partition_broadcast(rb[:], rinv[:])
                        ybase = (h % 2) * 64
                        nc.vector.tensor_mul(
                            out=yn[h // 2][ybase : ybase + 64, jq * TQC : (jq + 1) * TQC],
                            in0=pys[h][0:64, :],
                            in1=rb[:],
                        )
                emit_proj(NJQ - 1)

    nc.compile()
    return nc


def _host_tables():
    # RoPE tables in interleaved-pair device layout (row j'=2i <-> orig j=i,
    # j'=2i+1 <-> orig j=i+32); sign of the shuffled sin term folded in.
    inv = 1.0 / (10000.0 ** (np.arange(0, D, 2, dtype=np.float64) / D))  # (32,)
    t = np.arange(T, dtype=np.float64)
    fr = np.outer(t, inv)  # (T, 32)
    cos_h = np.cos(fr).T.astype(np.float32)  # (32, T)
    sin_h = np.sin(fr).T.astype(np.float32)
    cosI = np.empty((D, T), np.float32)
    sinI = np.empty((D, T), np.float32)
    cosI[0::2] = cos_h
    cosI[1::2] = cos_h
    sinI[0::2] = -sin_h
    sinI[1::2] = sin_h
    cos_q = np.tile(cosI, (2, 1))
    sin_q = np.tile(sinI, (2, 1))
    # masks: staircase tile s of a tq-chunk: allowed iff s*128 + tkl <= tql
    tkl = np.arange(128)[:, None]
    tql = np.arange(TQC)[None, :]
    masks = np.stack(
        [(s * 128 + tkl <= tql).astype(np.float32) for s in range(4)], axis=1
    ).astype(ml_dtypes.bfloat16)  # (128, 4, TQC)
    identb = np.tile(np.eye(64, dtype=np.float32), (2, 2))
    return cos_q, sin_q, masks, identb


def make_in_maps(x, wq, wk, wv, wproj):
    cos_q, sin_q, masks, identb = _host_tables()
    # interleave permutation within each head's 64 cols: perm[2i]=i, perm[2i+1]=i+32
    perm = np.empty(D, np.int64)
    perm[0::2] = np.arange(32)
    perm[1::2] = np.arange(32) + 32
    in_maps = []
    for c in range(8):
        b, h = c // 4, c % 4
        xT = np.ascontiguousarray(x[b].T)  # (C, T)
        wq_h = wq[:, h * 256 : (h + 1) * 256].reshape(C, G, D)[:, :, perm].reshape(C, 256)
        wk_h = wk[:, h * 64 : (h + 1) * 64][:, perm]
        wv_h = wv[:, h * 64 : (h + 1) * 64]
        w_all = np.ascontiguousarray(np.concatenate([wq_h, wk_h, wv_h], axis=1))
        wp_h = np.ascontiguousarray(wproj[h * 256 : (h + 1) * 256, :])
        in_maps.append(
            {
                "xT": xT,
                "w_all": w_all,
                "wp": wp_h,
                "cos_q": cos_q,
                "sin_q": sin_q,
                "masks": masks,
                "identb": identb,
            }
        )
    return in_maps


def kernel(x, wq, wk, wv, wproj):
    x = np.asarray(x, dtype=np.float32)
    wq = np.asarray(wq, dtype=np.float32)
    wk = np.asarray(wk, dtype=np.float32)
    wv = np.asarray(wv, dtype=np.float32)
    wproj = np.asarray(wproj, dtype=np.float32)
    B = x.shape[0]

    if "nc" not in _PROG:
        _PROG["nc"] = _build_program()
    nc = _PROG["nc"]

    in_maps = make_in_maps(x, wq, wk, wv, wproj)

    res = run_bass_kernel_spmd(nc, in_maps, list(range(8)))
    out = np.zeros((B, T, C), np.float32)
    for c in range(8):
        out[c // 4] += res.results[c]["out_p"]
    return out


# revision 8
# speedup vs baseline: 1.1033x; 1.0042x over previous
"""Causal self-attention (GQA + RoPE) Trainium2 kernel over 8 NeuronCores.

Sharding: 8 cores = batch(2) x kv_head(4). Each core computes its batch's
4 q-heads / 1 kv-head attention plus the partial output projection; host
sums the 4 partial projections per batch element.

Device algorithm (fully transposed "k-major" attention, zero P-transposes):
  stage A: QKV^T = [wq|wk|wv]^T @ x^T on PE (fp32r)
  RoPE on DVE via interleaved-pair layout (host permutes wq/wk columns so
    rotation partners are adjacent partitions -> stream_shuffle swap)
  V^T -> V via PE transposes; ones-column appended -> PV matmul emits both
    Y^T and softmax sums in one accumulation
  S^T = K^T.T @ Q^T per (tk-tile, tq-chunk); exp on ACT (no max subtraction,
    scores bounded); causal staircase masks multiplied on DVE
  normalize: reciprocal of sums + gpsimd partition_broadcast + DVE mul
  projection: wproj rows for this core's heads, fp32r; partial out to HBM
"""

import sys

sys.path.insert(0, "/opt/trn_rl_repo")

import numpy as np
import ml_dtypes

import concourse.bacc as bacc
import concourse.tile as tile
from concourse import mybir
from concourse.bass_utils import run_bass_kernel_spmd

F32 = mybir.dt.float32
F32R = mybir.dt.float32r
BF16 = mybir.dt.bfloat16
AF = mybir.ActivationFunctionType

T, C, D, H, HKV = 2048, 1024, 64, 16, 4
G = H // HKV  # q heads per kv head
NCC = C // 128  # 8 contraction chunks
NJQ = 4  # tq chunks of 512
TQC = 512
NTK = T // 128  # 16 tk tiles
SCALE = 1.0 / 8.0  # 1/sqrt(D)

_PROG = {}


def _build_program():
    nc = bacc.Bacc()
    xT_d = nc.dram_tensor("xT", [C, T], F32R, kind="ExternalInput")
    w_d = nc.dram_tensor("w_all", [C, 384], F32R, kind="ExternalInput")
    wp_d = nc.dram_tensor("wp", [256, C], F32R, kind="ExternalInput")
    cq_d = nc.dram_tensor("cos_q", [128, T], F32, kind="ExternalInput")
    sq_d = nc.dram_tensor("sin_q", [128, T], F32, kind="ExternalInput")
    mk_d = nc.dram_tensor("masks", [128, 4, TQC], BF16, kind="ExternalInput")
    id_d = nc.dram_tensor("identb", [128, 128], F32, kind="ExternalInput")
    out_d = nc.dram_tensor("out_p", [T, C], F32, kind="ExternalOutput")

    swap_mask = [i ^ 1 for i in range(32)]

    with tile.TileContext(nc) as tc:
        with (
            tc.tile_pool(name="const", bufs=1) as const,
            tc.tile_pool(name="big", bufs=1) as big,
        ):
            # ---- constants (W first so stage A starts ASAP; wp last) ----
            W_sb = const.tile([128, NCC, 384], F32R, tag="W", name="W_sb")
            nc.sync.dma_start(out=W_sb[:], in_=w_d[:].rearrange("(n p) m -> p n m", p=128))
            cq_sb = const.tile([128, T], F32, tag="cq", name="cq_sb")
            sq_sb = const.tile([128, T], F32, tag="sq", name="sq_sb")
            mk_sb = const.tile([128, 4, TQC], BF16, tag="mk", name="mk_sb")
            id_sb = const.tile([128, 128], F32, tag="idb", name="id_sb")
            wp_sb = const.tile([128, 2, C], F32R, tag="wp", name="wp_sb")

            qkv_sb = [big.tile([128, T], F32, tag=f"qkv{m}", name=f"qkv{m}") for m in range(3)]
            qrope = [big.tile([128, T], BF16, tag=f"qr{m}", name=f"qr{m}") for m in range(2)]
            k2 = big.tile([128, T], BF16, tag="k2", name="k2")
            vhat = big.tile([128, NTK, 65], BF16, tag="vhat", name="vhat")
            yn = [big.tile([128, T], F32R, tag=f"yn{m}", name=f"yn{m}") for m in range(2)]

            # ---- stage A: QKV^T = W^T @ x^T (fp32r) + V transposes ----
            with (
                tc.tile_pool(name="xp", bufs=1) as xp,
                tc.tile_pool(name="rope", bufs=1) as ropep,
                tc.tile_pool(name="psA", bufs=1, space="PSUM") as psA,
                tc.tile_pool(name="psT", bufs=2, space="PSUM") as psT,
            ):
                xts = []
                for cc in range(NCC):
                    xt = xp.tile([128, T], F32R, tag=f"x{cc}", name=f"x{cc}")
                    nc.sync.dma_start(out=xt[:], in_=xT_d[cc * 128 : (cc + 1) * 128, :])
                    xts.append(xt)
                nc.sync.dma_start(out=cq_sb[:], in_=cq_d[:])
                nc.sync.dma_start(out=sq_sb[:], in_=sq_d[:])
                nc.sync.dma_start(out=mk_sb[:], in_=mk_d[:])
                nc.sync.dma_start(out=id_sb[:], in_=id_d[:])
                nc.sync.dma_start(out=wp_sb[:], in_=wp_d[:].rearrange("(n p) m -> p n m", p=128))

                for mt in (2, 0, 1):
                    pas = [psA.tile([128, TQC], F32, tag=f"pa{j}", name=f"pa{j}") for j in range(NJQ)]
                    for cc in range(NCC):
                        for jq in range(NJQ):
                            nc.tensor.matmul(
                                pas[jq][:],
                                lhsT=W_sb[:, cc, mt * 128 : (mt + 1) * 128],
                                rhs=xts[cc][:, jq * TQC : (jq + 1) * TQC],
                                start=(cc == 0),
                                stop=(cc == NCC - 1),
                            )
                    for jq in range(NJQ):
                        nc.scalar.copy(
                            out=qkv_sb[mt][:, jq * TQC : (jq + 1) * TQC], in_=pas[jq][:]
                        )

                # RoPE: interleaved-pair rotation, sign/scale folded into tables
                for pt in (2, 0, 1):
                    rows = 128 if pt < 2 else 64
                    ct = cq_sb
                    st = sq_sb
                    dst = qrope[pt] if pt < 2 else k2
                    src = qkv_sb[pt]
                    shuf = ropep.tile([128, T], F32, tag="shuf", name="shuf")
                    prod = ropep.tile([128, T], F32, tag="prod", name="prod")
                    nc.vector.stream_shuffle(shuf[:rows, :], src[:rows, :], mask=swap_mask)
                    nc.vector.tensor_mul(out=shuf[:rows, :], in0=shuf[:rows, :], in1=st[:rows, :])
                    nc.vector.tensor_mul(out=prod[:rows, :], in0=src[:rows, :], in1=ct[:rows, :])
                    nc.vector.tensor_add(out=dst[:rows, :], in0=prod[:rows, :], in1=shuf[:rows, :])

                # duplicate K^T into partitions 64:128 (pairs heads for PE row groups)
                nc.sync.dma_start(out=k2[64:128, :], in_=k2[0:64, :])

                # Vhat: V (t-major) + ones column for softmax sums
                nc.vector.memset(vhat[:, :, 64:65], 1.0)
                for tt in range(NTK):
                    pt_ = psT.tile([128, 64], F32, tag="ptr", name="ptr")
                    nc.tensor.transpose(
                        pt_[:],
                        qkv_sb[2][64:128, tt * 128 : (tt + 1) * 128],
                        id_sb[64:128, 0:64],
                    )
                    nc.vector.tensor_copy(out=vhat[:, tt, 0:64], in_=pt_[:])

            # ---- attention + projection ----
            with (
                tc.tile_pool(name="ptiles", bufs=6) as ppool,
                tc.tile_pool(name="small", bufs=3) as small,
                tc.tile_pool(name="outp", bufs=3) as outp,
                tc.tile_pool(name="psS", bufs=2, space="PSUM") as psS,
                tc.tile_pool(name="psY", bufs=4, space="PSUM") as psY,
                tc.tile_pool(name="psP", bufs=1, space="PSUM") as psP,
            ):

                def emit_proj(pjq):
                    for tt in range(4 * pjq, 4 * pjq + 4):
                        outsb = outp.tile([128, C], F32, tag="osb", name="osb")
                        pps = [psP.tile([128, 512], F32, tag=f"pp{n}", name=f"pp{n}") for n in range(2)]
                        for kk in range(2):
                            for ncol in range(2):
                                nc.tensor.matmul(
                                    pps[ncol][:],
                                    lhsT=yn[kk][:, tt * 128 : (tt + 1) * 128],
                                    rhs=wp_sb[:, kk, ncol * 512 : (ncol + 1) * 512],
                                    start=(kk == 0),
                                    stop=(kk == 1),
                                )
                        for ncol in range(2):
                            nc.vector.tensor_copy(
                                out=outsb[:, ncol * 512 : (ncol + 1) * 512], in_=pps[ncol][:]
                            )
                        nc.sync.dma_start(
                            out=out_d[tt * 128 : (tt + 1) * 128, :], in_=outsb[:]
                        )

                for jq in range(NJQ):
                    nik = 4 * jq + 4
                    pys = [psY.tile([65, TQC], F32, tag="py", name="py") for _ in range(4)]
                    for ik in range(nik):
                        s = ik - 4 * jq
                        lo = max(s, 0) * 128  # staircase: cols < lo are fully masked
                        sss, ppt = [], []
                        # S phase: order h0,h2 then h1,h3 so consecutive MMs share lhsT
                        for h in (0, 2, 1, 3):
                            qt = qrope[h // 2]
                            base = (h % 2) * 64
                            ps_s = psS.tile([128, TQC], F32, tag="ps_s", name="ps_s")
                            nc.tensor.matmul(
                                ps_s[:, lo:TQC],
                                lhsT=k2[base : base + 64, ik * 128 : (ik + 1) * 128],
                                rhs=qt[base : base + 64, jq * TQC + lo : (jq + 1) * TQC],
                                start=True,
                                stop=True,
                            )
                            sss.append(ps_s)
                        for i, h in enumerate((0, 2, 1, 3)):
                            ptile = ppool.tile([128, TQC], BF16, tag="pt", name="ptile")
                            nc.scalar.activation(
                                out=ptile[:, lo:TQC], in_=sss[i][:, lo:TQC], func=AF.Exp, scale=SCALE
                            )
                            if s >= 0:
                                # only the diagonal 128-block needs the tri mask
                                nc.vector.tensor_mul(
                                    out=ptile[:, lo : lo + 128],
                                    in0=ptile[:, lo : lo + 128],
                                    in1=mk_sb[:, 0, 0:128],
                                )
                            ppt.append(ptile)
                        # PV phase: all 4 share the vhat lhsT
                        for i, h in enumerate((0, 2, 1, 3)):
                            nc.tensor.matmul(
                                pys[h][:, lo:TQC],
                                lhsT=vhat[:, ik, :],
                                rhs=ppt[i][:, lo:TQC],
                                start=(ik == 0),
                                stop=(ik == nik - 1),
                            )
                    # projection for the PREVIOUS chunk's t-tiles: deferred one
                    # chunk so the in-order PE stream never stalls on the
                    # normalize chain (its yn inputs finished long ago).
                    if jq > 0:
                        emit_proj(jq - 1)
                    for h in range(4):
                        srow = small.tile([1, TQC], F32, tag="srow", name="srow")
                        nc.vector.tensor_copy(out=srow[:], in_=pys[h][64:65, :])
                        rinv = small.tile([1, TQC], F32, tag="rinv", name="rinv")
                        nc.vector.reciprocal_approx_fast(out=rinv[:], in_=srow[:])
                        rb = small.tile([64, TQC], F32, tag="rb", name="rb")
                        nc.gpsimd.partition_broadcast(rb[:], rinv[:])
                        ybase = (h % 2) * 64
                        nc.vector.tensor_mul(
                            out=yn[h // 2][ybase : ybase + 64, jq * TQC : (jq + 1) * TQC],
                            in0=pys[h][0:64, :],
                            in1=rb[:],
                        )
                emit_proj(NJQ - 1)

    nc.compile()
    return nc


def _host_tables():
    # RoPE tables in interleaved-pair device layout (row j'=2i <-> orig j=i,
    # j'=2i+1 <-> orig j=i+32); sign of the shuffled sin term folded in.
    inv = 1.0 / (10000.0 ** (np.arange(0, D, 2, dtype=np.float64) / D))  # (32,)
    t = np.arange(T, dtype=np.float64)
    fr = np.outer(t, inv)  # (T, 32)
    cos_h = np.cos(fr).T.astype(np.float32)  # (32, T)
    sin_h = np.sin(fr).T.astype(np.float32)
    cosI = np.empty((D, T), np.float32)
    sinI = np.empty((D, T), np.float32)
    cosI[0::2] = cos_h
    cosI[1::2] = cos_h
    sinI[0::2] = -sin_h
    sinI[1::2] = sin_h
    cos_q = np.tile(cosI, (2, 1))
    sin_q = np.tile(sinI, (2, 1))
    # masks: staircase tile s of a tq-chunk: allowed iff s*128 + tkl <= tql
    tkl = np.arange(128)[:, None]
    tql = np.arange(TQC)[None, :]
    masks = np.stack(
        [(s * 128 + tkl <= tql).astype(np.float32) for s in range(4)], axis=1
    ).astype(ml_dtypes.bfloat16)  # (128, 4, TQC)
    identb = np.tile(np.eye(64, dtype=np.float32), (2, 2))
    return cos_q, sin_q, masks, identb


def make_in_maps(x, wq, wk, wv, wproj):
    cos_q, sin_q, masks, identb = _host_tables()
    # interleave permutation within each head's 64 cols: perm[2i]=i, perm[2i+1]=i+32
    perm = np.empty(D, np.int64)
    perm[0::2] = np.arange(32)
    perm[1::2] = np.arange(32) + 32
    in_maps = []
    for c in range(8):
        b, h = c // 4, c % 4
        xT = np.ascontiguousarray(x[b].T)  # (C, T)
        wq_h = wq[:, h * 256 : (h + 1) * 256].reshape(C, G, D)[:, :, perm].reshape(C, 256)
        wk_h = wk[:, h * 64 : (h + 1) * 64][:, perm]
        wv_h = wv[:, h * 64 : (h + 1) * 64]
        w_all = np.ascontiguousarray(np.concatenate([wq_h, wk_h, wv_h], axis=1))
        wp_h = np.ascontiguousarray(wproj[h * 256 : (h + 1) * 256, :])
        in_maps.append(
            {
                "xT": xT,
                "w_all": w_all,
                "wp": wp_h,
                "cos_q": cos_q,
                "sin_q": sin_q,
                "masks": masks,
                "identb": identb,
            }
        )
    return in_maps


def kernel(x, wq, wk, wv, wproj):
    x = np.asarray(x, dtype=np.float32)
    wq = np.asarray(wq, dtype=np.float32)
    wk = np.asarray(wk, dtype=np.float32)
    wv = np.asarray(wv, dtype=np.float32)
    wproj = np.asarray(wproj, dtype=np.float32)
    B = x.shape[0]

    if "nc" not in _PROG:
        _PROG["nc"] = _build_program()
    nc = _PROG["nc"]

    in_maps = make_in_maps(x, wq, wk, wv, wproj)

    res = run_bass_kernel_spmd(nc, in_maps, list(range(8)))
    out = np.zeros((B, T, C), np.float32)
    for c in range(8):
        out[c // 4] += res.results[c]["out_p"]
    return out


# revision 10
# speedup vs baseline: 1.2839x; 1.1637x over previous
"""Causal self-attention (GQA + RoPE) Trainium2 kernel over 8 NeuronCores.

Sharding: 8 cores = batch(2) x kv_head(4). Each core computes its batch's
4 q-heads / 1 kv-head attention plus the partial output projection; host
sums the 4 partial projections per batch element.

Device algorithm (fully transposed "k-major" attention, zero P-transposes):
  stage A: QKV^T = [wq|wk|wv]^T @ x^T on PE (fp32r)
  RoPE on DVE via interleaved-pair layout (host permutes wq/wk columns so
    rotation partners are adjacent partitions -> stream_shuffle swap)
  V^T -> V via PE transposes; ones-column appended -> PV matmul emits both
    Y^T and softmax sums in one accumulation
  S^T = K^T.T @ Q^T per (tk-tile, tq-chunk); exp on ACT (no max subtraction,
    scores bounded); causal staircase masks multiplied on DVE
  normalize: reciprocal of sums + gpsimd partition_broadcast + DVE mul
  projection: wproj rows for this core's heads, fp32r; partial out to HBM
"""

import sys

sys.path.insert(0, "/opt/trn_rl_repo")

import numpy as np
import ml_dtypes

import concourse.bacc as bacc
import concourse.tile as tile
from concourse import mybir
from concourse.bass_utils import run_bass_kernel_spmd

F32 = mybir.dt.float32
F32R = mybir.dt.float32r
BF16 = mybir.dt.bfloat16
AF = mybir.ActivationFunctionType

T, C, D, H, HKV = 2048, 1024, 64, 16, 4
G = H // HKV  # q heads per kv head
NCC = C // 128  # 8 contraction chunks
NJQ = 4  # tq chunks of 512
TQC = 512
NTK = T // 128  # 16 tk tiles
SCALE = 1.0 / 8.0  # 1/sqrt(D)

_PROG = {}


def _build_program():
    nc = bacc.Bacc()
    xT_d = nc.dram_tensor("xT", [C, T], F32R, kind="ExternalInput")
    w_d = nc.dram_tensor("w_all", [C, 384], F32R, kind="ExternalInput")
    wp_d = nc.dram_tensor("wp", [256, C], F32R, kind="ExternalInput")
    cq_d = nc.dram_tensor("cos_q", [128, T], F32, kind="ExternalInput")
    sq_d = nc.dram_tensor("sin_q", [128, T], F32, kind="ExternalInput")
    mk_d = nc.dram_tensor("masks", [128, 4, TQC], BF16, kind="ExternalInput")
    id_d = nc.dram_tensor("identb", [128, 128], F32, kind="ExternalInput")
    out_d = nc.dram_tensor("out_p", [T, C], F32, kind="ExternalOutput")

    swap_mask = [i ^ 1 for i in range(32)]

    with tile.TileContext(nc) as tc:
        with (
            tc.tile_pool(name="const", bufs=1) as const,
            tc.tile_pool(name="big", bufs=1) as big,
        ):
            # ---- constants (W first so stage A starts ASAP; wp last) ----
            W_sb = const.tile([128, NCC, 384], F32R, tag="W", name="W_sb")
            nc.sync.dma_start(out=W_sb[:], in_=w_d[:].rearrange("(n p) m -> p n m", p=128))
            cq_sb = const.tile([128, T], F32, tag="cq", name="cq_sb")
            sq_sb = const.tile([128, T], F32, tag="sq", name="sq_sb")
            mk_sb = const.tile([128, 4, TQC], BF16, tag="mk", name="mk_sb")
            id_sb = const.tile([128, 128], F32, tag="idb", name="id_sb")
            wp_sb = const.tile([128, 2, C], F32R, tag="wp", name="wp_sb")

            qkv_sb = [big.tile([128, T], F32, tag=f"qkv{m}", name=f"qkv{m}") for m in range(3)]
            qrope = [big.tile([128, T], BF16, tag=f"qr{m}", name=f"qr{m}") for m in range(2)]
            k2 = big.tile([128, T], BF16, tag="k2", name="k2")
            vhat = big.tile([128, NTK, 65], BF16, tag="vhat", name="vhat")
            yn = [big.tile([128, T], F32R, tag=f"yn{m}", name=f"yn{m}") for m in range(2)]

            # ---- stage A: QKV^T = W^T @ x^T (fp32r) + V transposes ----
            with (
                tc.tile_pool(name="xp", bufs=1) as xp,
                tc.tile_pool(name="rope", bufs=1) as ropep,
                tc.tile_pool(name="psA", bufs=1, space="PSUM") as psA,
                tc.tile_pool(name="psT", bufs=2, space="PSUM") as psT,
            ):
                xts = []
                for cc in range(NCC):
                    xt = xp.tile([128, T], F32R, tag=f"x{cc}", name=f"x{cc}")
                    nc.sync.dma_start(out=xt[:], in_=xT_d[cc * 128 : (cc + 1) * 128, :])
                    xts.append(xt)
                nc.sync.dma_start(out=cq_sb[:], in_=cq_d[:])
                nc.sync.dma_start(out=sq_sb[:], in_=sq_d[:])
                nc.sync.dma_start(out=mk_sb[:], in_=mk_d[:])
                nc.sync.dma_start(out=id_sb[:], in_=id_d[:])
                nc.sync.dma_start(out=wp_sb[:], in_=wp_d[:].rearrange("(n p) m -> p n m", p=128))

                for mt in (2, 0, 1):
                    pas = [psA.tile([128, TQC], F32, tag=f"pa{j}", name=f"pa{j}") for j in range(NJQ)]
                    for cc in range(NCC):
                        for jq in range(NJQ):
                            nc.tensor.matmul(
                                pas[jq][:],
                                lhsT=W_sb[:, cc, mt * 128 : (mt + 1) * 128],
                                rhs=xts[cc][:, jq * TQC : (jq + 1) * TQC],
                                start=(cc == 0),
                                stop=(cc == NCC - 1),
                            )
                    for jq in range(NJQ):
                        nc.scalar.copy(
                            out=qkv_sb[mt][:, jq * TQC : (jq + 1) * TQC], in_=pas[jq][:]
                        )

                # RoPE: interleaved-pair rotation, sign/scale folded into tables
                for pt in (2, 0, 1):
                    rows = 128 if pt < 2 else 64
                    ct = cq_sb
                    st = sq_sb
                    dst = qrope[pt] if pt < 2 else k2
                    src = qkv_sb[pt]
                    shuf = ropep.tile([128, T], F32, tag="shuf", name="shuf")
                    prod = ropep.tile([128, T], F32, tag="prod", name="prod")
                    nc.vector.stream_shuffle(shuf[:rows, :], src[:rows, :], mask=swap_mask)
                    nc.vector.tensor_mul(out=shuf[:rows, :], in0=shuf[:rows, :], in1=st[:rows, :])
                    nc.vector.tensor_mul(out=prod[:rows, :], in0=src[:rows, :], in1=ct[:rows, :])
                    nc.vector.tensor_add(out=dst[:rows, :], in0=prod[:rows, :], in1=shuf[:rows, :])

                # duplicate K^T into partitions 64:128 (pairs heads for PE row groups)
                nc.sync.dma_start(out=k2[64:128, :], in_=k2[0:64, :])

                # Vhat: V (t-major) + ones column for softmax sums
                nc.vector.memset(vhat[:, :, 64:65], 1.0)
                for tt in range(NTK):
                    pt_ = psT.tile([128, 64], F32, tag="ptr", name="ptr")
                    nc.tensor.transpose(
                        pt_[:],
                        qkv_sb[2][64:128, tt * 128 : (tt + 1) * 128],
                        id_sb[64:128, 0:64],
                    )
                    nc.vector.tensor_copy(out=vhat[:, tt, 0:64], in_=pt_[:])

            # ---- attention + projection ----
            with (
                tc.tile_pool(name="ptiles", bufs=6) as ppool,
                tc.tile_pool(name="small", bufs=3) as small,
                tc.tile_pool(name="outp", bufs=3) as outp,
                tc.tile_pool(name="psS", bufs=2, space="PSUM") as psS,
                tc.tile_pool(name="psY", bufs=2, space="PSUM") as psY,
                tc.tile_pool(name="psP", bufs=1, space="PSUM") as psP,
            ):

                def emit_proj(pjq):
                    for tt in range(4 * pjq, 4 * pjq + 4):
                        outsb = outp.tile([128, C], F32, tag="osb", name="osb")
                        pps = [psP.tile([128, 512], F32, tag=f"pp{n}", name=f"pp{n}") for n in range(2)]
                        for kk in range(2):
                            for ncol in range(2):
                                nc.tensor.matmul(
                                    pps[ncol][:],
                                    lhsT=yn[kk][:, tt * 128 : (tt + 1) * 128],
                                    rhs=wp_sb[:, kk, ncol * 512 : (ncol + 1) * 512],
                                    start=(kk == 0),
                                    stop=(kk == 1),
                                )
                        for ncol in range(2):
                            nc.vector.tensor_copy(
                                out=outsb[:, ncol * 512 : (ncol + 1) * 512], in_=pps[ncol][:]
                            )
                        nc.sync.dma_start(
                            out=out_d[tt * 128 : (tt + 1) * 128, :], in_=outsb[:]
                        )

                # Two head-pair passes: pass 0 = heads {0,1} (fills yn[0]),
                # pass 1 = heads {2,3} (fills yn[1]) with projection for each
                # chunk interleaved once both passes have covered it.
                # S tiles for consecutive ik pairs share one PSUM group tile so
                # a single ACTIVATE exps 1024 columns (exp is elementwise; the
                # per-ik partition semantics only matter to the PV consumer).
                for hp in range(2):
                    for jq in range(NJQ):
                        nik = 4 * jq + 4
                        pys = [psY.tile([65, TQC], F32, tag="py", name="py") for _ in range(2)]
                        for ika in range(0, nik, 2):
                            iks = (ika, ika + 1)
                            for hh in range(2):
                                h = 2 * hp + hh
                                qt = qrope[hp]
                                base = hh * 64
                                ps_g = psS.tile([128, 2, TQC], F32, tag="ps_g", name="ps_g")
                                los = []
                                for gi, ik in enumerate(iks):
                                    s = ik - 4 * jq
                                    lo = max(s, 0) * 128
                                    los.append(lo)
                                    nc.tensor.matmul(
                                        ps_g[:, gi, lo:TQC],
                                        lhsT=k2[base : base + 64, ik * 128 : (ik + 1) * 128],
                                        rhs=qt[base : base + 64, jq * TQC + lo : (jq + 1) * TQC],
                                        start=True,
                                        stop=True,
                                    )
                                ptile = ppool.tile([128, 2, TQC], BF16, tag="pt", name="ptile")
                                nc.scalar.activation(
                                    out=ptile[:], in_=ps_g[:], func=AF.Exp, scale=SCALE
                                )
                                for gi, ik in enumerate(iks):
                                    if ik - 4 * jq >= 0:
                                        lo = los[gi]
                                        nc.vector.tensor_mul(
                                            out=ptile[:, gi, lo : lo + 128],
                                            in0=ptile[:, gi, lo : lo + 128],
                                            in1=mk_sb[:, 0, 0:128],
                                        )
                                for gi, ik in enumerate(iks):
                                    lo = los[gi]
                                    nc.tensor.matmul(
                                        pys[hh][:, lo:TQC],
                                        lhsT=vhat[:, ik, :],
                                        rhs=ptile[:, gi, lo:TQC],
                                        start=(ik == 0),
                                        stop=(ik == nik - 1),
                                    )
                        # proj for the chunk this pass completed one step ago
                        if hp == 1 and jq > 0:
                            emit_proj(jq - 1)
                        for hh in range(2):
                            h = 2 * hp + hh
                            srow = small.tile([1, TQC], F32, tag="srow", name="srow")
                            nc.vector.tensor_copy(out=srow[:], in_=pys[hh][64:65, :])
                            rinv = small.tile([1, TQC], F32, tag="rinv", name="rinv")
                            nc.vector.reciprocal_approx_fast(out=rinv[:], in_=srow[:])
                            rb = small.tile([64, TQC], F32, tag="rb", name="rb")
                            nc.gpsimd.partition_broadcast(rb[:], rinv[:])
                            nc.vector.tensor_mul(
                                out=yn[hp][hh * 64 : hh * 64 + 64, jq * TQC : (jq + 1) * TQC],
                                in0=pys[hh][0:64, :],
                                in1=rb[:],
                            )
                emit_proj(NJQ - 1)

    nc.compile()
    return nc


def _host_tables():
    # RoPE tables in interleaved-pair device layout (row j'=2i <-> orig j=i,
    # j'=2i+1 <-> orig j=i+32); sign of the shuffled sin term folded in.
    inv = 1.0 / (10000.0 ** (np.arange(0, D, 2, dtype=np.float64) / D))  # (32,)
    t = np.arange(T, dtype=np.float64)
    fr = np.outer(t, inv)  # (T, 32)
    cos_h = np.cos(fr).T.astype(np.float32)  # (32, T)
    sin_h = np.sin(fr).T.astype(np.float32)
    cosI = np.empty((D, T), np.float32)
    sinI = np.empty((D, T), np.float32)
    cosI[0::2] = cos_h
    cosI[1::2] = cos_h
    sinI[0::2] = -sin_h
    sinI[1::2] = sin_h
    cos_q = np.tile(cosI, (2, 1))
    sin_q = np.tile(sinI, (2, 1))
    # masks: staircase tile s of a tq-chunk: allowed iff s*128 + tkl <= tql
    tkl = np.arange(128)[:, None]
    tql = np.arange(TQC)[None, :]
    masks = np.stack(
        [(s * 128 + tkl <= tql).astype(np.float32) for s in range(4)], axis=1
    ).astype(ml_dtypes.bfloat16)  # (128, 4, TQC)
    identb = np.tile(np.eye(64, dtype=np.float32), (2, 2))
    return cos_q, sin_q, masks, identb


def make_in_maps(x, wq, wk, wv, wproj):
    cos_q, sin_q, masks, identb = _host_tables()
    # interleave permutation within each head's 64 cols: perm[2i]=i, perm[2i+1]=i+32
    perm = np.empty(D, np.int64)
    perm[0::2] = np.arange(32)
    perm[1::2] = np.arange(32) + 32
    in_maps = []
    for c in range(8):
        b, h = c // 4, c % 4
        xT = np.ascontiguousarray(x[b].T)  # (C, T)
        wq_h = wq[:, h * 256 : (h + 1) * 256].reshape(C, G, D)[:, :, perm].reshape(C, 256)
        wk_h = wk[:, h * 64 : (h + 1) * 64][:, perm]
        wv_h = wv[:, h * 64 : (h + 1) * 64]
        w_all = np.ascontiguousarray(np.concatenate([wq_h, wk_h, wv_h], axis=1))
        wp_h = np.ascontiguousarray(wproj[h * 256 : (h + 1) * 256, :])
        in_maps.append(
            {
                "xT": xT,
                "w_all": w_all,
                "wp": wp_h,
                "cos_q": cos_q,
                "sin_q": sin_q,
                "masks": masks,
                "identb": identb,
            }
        )
    return in_maps


def kernel(x, wq, wk, wv, wproj):
    x = np.asarray(x, dtype=np.float32)
    wq = np.asarray(wq, dtype=np.float32)
    wk = np.asarray(wk, dtype=np.float32)
    wv = np.asarray(wv, dtype=np.float32)
    wproj = np.asarray(wproj, dtype=np.float32)
    B = x.shape[0]

    if "nc" not in _PROG:
        _PROG["nc"] = _build_program()
    nc = _PROG["nc"]

    in_maps = make_in_maps(x, wq, wk, wv, wproj)

    res = run_bass_kernel_spmd(nc, in_maps, list(range(8)))
    out = np.zeros((B, T, C), np.float32)
    for c in range(8):
        out[c // 4] += res.results[c]["out_p"]
    return out


# revision 12
# speedup vs baseline: 1.5298x; 1.1916x over previous
"""Causal self-attention (GQA + RoPE) Trainium2 kernel over 8 NeuronCores.

Sharding: 8 cores = batch(2) x kv_head(4). Each core computes its batch's
4 q-heads / 1 kv-head attention plus the partial output projection; host
sums the 4 partial projections per batch element.

Device algorithm (fully transposed "k-major" attention, zero P-transposes):
  stage A: QKV^T = [wq|wk|wv]^T @ x^T on PE (fp32r)
  RoPE on DVE via interleaved-pair layout (host permutes wq/wk columns so
    rotation partners are adjacent partitions -> stream_shuffle swap)
  V^T -> V via PE transposes; ones-column appended -> PV matmul emits both
    Y^T and softmax sums in one accumulation
  S^T = K^T.T @ Q^T per (tk-tile, tq-chunk); exp on ACT (no max subtraction,
    scores bounded); causal staircase masks multiplied on DVE
  normalize: reciprocal of sums + gpsimd partition_broadcast + DVE mul
  projection: wproj rows for this core's heads, fp32r; partial out to HBM
"""

import sys

sys.path.insert(0, "/opt/trn_rl_repo")

import numpy as np
import ml_dtypes

import concourse.bacc as bacc
import concourse.tile as tile
from concourse import mybir
from concourse.bass_utils import run_bass_kernel_spmd

F32 = mybir.dt.float32
F32R = mybir.dt.float32r
BF16 = mybir.dt.bfloat16
AF = mybir.ActivationFunctionType

T, C, D, H, HKV = 2048, 1024, 64, 16, 4
G = H // HKV  # q heads per kv head
NCC = C // 128  # 8 contraction chunks
NJQ = 4  # tq chunks of 512
TQC = 512
NTK = T // 128  # 16 tk tiles
SCALE = 1.0 / 8.0  # 1/sqrt(D)

_PROG = {}


def _build_program():
    nc = bacc.Bacc()
    xT_d = nc.dram_tensor("xT", [C, T], BF16, kind="ExternalInput")
    w_d = nc.dram_tensor("w_all", [C, 384], BF16, kind="ExternalInput")
    wp_d = nc.dram_tensor("wp", [256, C], BF16, kind="ExternalInput")
    cq_d = nc.dram_tensor("cos_q", [128, T], F32, kind="ExternalInput")
    sq_d = nc.dram_tensor("sin_q", [128, T], F32, kind="ExternalInput")
    mk_d = nc.dram_tensor("masks", [128, 4, TQC], BF16, kind="ExternalInput")
    id_d = nc.dram_tensor("identb", [128, 128], F32, kind="ExternalInput")
    out_d = nc.dram_tensor("out_p", [T, C], F32, kind="ExternalOutput")

    swap_mask = [i ^ 1 for i in range(32)]

    with tile.TileContext(nc) as tc:
        with (
            tc.tile_pool(name="const", bufs=1) as const,
            tc.tile_pool(name="big", bufs=1) as big,
        ):
            # ---- constants (W first so stage A starts ASAP; wp last) ----
            W_sb = const.tile([128, NCC, 384], BF16, tag="W", name="W_sb")
            cq_sb = const.tile([128, T], F32, tag="cq", name="cq_sb")
            sq_sb = const.tile([128, T], F32, tag="sq", name="sq_sb")
            mk_sb = const.tile([128, 4, TQC], BF16, tag="mk", name="mk_sb")
            id_sb = const.tile([128, 128], F32, tag="idb", name="id_sb")
            wp_sb = const.tile([128, 2, C], BF16, tag="wp", name="wp_sb")

            qkv_sb = [big.tile([128, T], F32, tag=f"qkv{m}", name=f"qkv{m}") for m in range(3)]
            qrope = [big.tile([128, T], BF16, tag=f"qr{m}", name=f"qr{m}") for m in range(2)]
            k2 = big.tile([128, T], BF16, tag="k2", name="k2")
            vhat = big.tile([128, NTK, 65], BF16, tag="vhat", name="vhat")
            yn = [big.tile([128, T], BF16, tag=f"yn{m}", name=f"yn{m}") for m in range(2)]

            # ---- stage A: QKV^T = W^T @ x^T (fp32r) + V transposes ----
            with (
                tc.tile_pool(name="xp", bufs=1) as xp,
                tc.tile_pool(name="rope", bufs=1) as ropep,
                tc.tile_pool(name="psA", bufs=1, space="PSUM") as psA,
                tc.tile_pool(name="psT", bufs=2, space="PSUM") as psT,
            ):
                xts = []
                for cc in range(NCC):
                    xt = xp.tile([128, T], BF16, tag=f"x{cc}", name=f"x{cc}")
                    nc.sync.dma_start(out=xt[:], in_=xT_d[cc * 128 : (cc + 1) * 128, :])
                    nc.sync.dma_start(
                        out=W_sb[:, cc, :], in_=w_d[cc * 128 : (cc + 1) * 128, :]
                    )
                    xts.append(xt)
                nc.sync.dma_start(out=cq_sb[:], in_=cq_d[:])
                nc.sync.dma_start(out=sq_sb[:], in_=sq_d[:])
                nc.sync.dma_start(out=mk_sb[:], in_=mk_d[:])
                nc.sync.dma_start(out=id_sb[:], in_=id_d[:])
                nc.sync.dma_start(out=wp_sb[:], in_=wp_d[:].rearrange("(n p) m -> p n m", p=128))

                for mt in (2, 0, 1):
                    pas = [psA.tile([128, TQC], F32, tag=f"pa{j}", name=f"pa{j}") for j in range(NJQ)]
                    for cc in range(NCC):
                        for jq in range(NJQ):
                            nc.tensor.matmul(
                                pas[jq][:],
                                lhsT=W_sb[:, cc, mt * 128 : (mt + 1) * 128],
                                rhs=xts[cc][:, jq * TQC : (jq + 1) * TQC],
                                start=(cc == 0),
                                stop=(cc == NCC - 1),
                            )
                    for jq in range(NJQ):
                        nc.scalar.copy(
                            out=qkv_sb[mt][:, jq * TQC : (jq + 1) * TQC], in_=pas[jq][:]
                        )

                # RoPE: interleaved-pair rotation, sign/scale folded into tables
                for pt in (2, 0, 1):
                    rows = 128 if pt < 2 else 64
                    ct = cq_sb
                    st = sq_sb
                    dst = qrope[pt] if pt < 2 else k2
                    src = qkv_sb[pt]
                    shuf = ropep.tile([128, T], F32, tag="shuf", name="shuf")
                    prod = ropep.tile([128, T], F32, tag="prod", name="prod")
                    nc.vector.stream_shuffle(shuf[:rows, :], src[:rows, :], mask=swap_mask)
                    nc.vector.tensor_mul(out=shuf[:rows, :], in0=shuf[:rows, :], in1=st[:rows, :])
                    nc.vector.tensor_mul(out=prod[:rows, :], in0=src[:rows, :], in1=ct[:rows, :])
                    nc.vector.tensor_add(out=dst[:rows, :], in0=prod[:rows, :], in1=shuf[:rows, :])

                # duplicate K^T into partitions 64:128 (pairs heads for PE row groups)
                nc.sync.dma_start(out=k2[64:128, :], in_=k2[0:64, :])

                # Vhat: V (t-major) + ones column for softmax sums
                nc.vector.memset(vhat[:, :, 64:65], 1.0)
                for tt in range(NTK):
                    pt_ = psT.tile([128, 64], F32, tag="ptr", name="ptr")
                    nc.tensor.transpose(
                        pt_[:],
                        qkv_sb[2][64:128, tt * 128 : (tt + 1) * 128],
                        id_sb[64:128, 0:64],
                    )
                    nc.vector.tensor_copy(out=vhat[:, tt, 0:64], in_=pt_[:])

            # ---- attention + projection ----
            with (
                tc.tile_pool(name="ptiles", bufs=6) as ppool,
                tc.tile_pool(name="small", bufs=3) as small,
                tc.tile_pool(name="outp", bufs=3) as outp,
                tc.tile_pool(name="psS", bufs=2, space="PSUM") as psS,
                tc.tile_pool(name="psY", bufs=2, space="PSUM") as psY,
                tc.tile_pool(name="psP", bufs=1, space="PSUM") as psP,
            ):

                def emit_proj(pjq):
                    for tt in range(4 * pjq, 4 * pjq + 4):
                        outsb = outp.tile([128, C], F32, tag="osb", name="osb")
                        pps = [psP.tile([128, 512], F32, tag=f"pp{n}", name=f"pp{n}") for n in range(2)]
                        for kk in range(2):
                            for ncol in range(2):
                                nc.tensor.matmul(
                                    pps[ncol][:],
                                    lhsT=yn[kk][:, tt * 128 : (tt + 1) * 128],
                                    rhs=wp_sb[:, kk, ncol * 512 : (ncol + 1) * 512],
                                    start=(kk == 0),
                                    stop=(kk == 1),
                                )
                        for ncol in range(2):
                            nc.vector.tensor_copy(
                                out=outsb[:, ncol * 512 : (ncol + 1) * 512], in_=pps[ncol][:]
                            )
                        nc.sync.dma_start(
                            out=out_d[tt * 128 : (tt + 1) * 128, :], in_=outsb[:]
                        )

                # Two head-pair passes: pass 0 = heads {0,1} (fills yn[0]),
                # pass 1 = heads {2,3} (fills yn[1]) with projection for each
                # chunk interleaved once both passes have covered it.
                # S tiles for consecutive ik pairs share one PSUM group tile so
                # a single ACTIVATE exps 1024 columns (exp is elementwise; the
                # per-ik partition semantics only matter to the PV consumer).
                for hp in range(2):
                    for jq in range(NJQ):
                        nik = 4 * jq + 4
                        pys = [psY.tile([65, TQC], F32, tag="py", name="py") for _ in range(2)]
                        for ika in range(0, nik, 2):
                            iks = (ika, ika + 1)
                            for hh in range(2):
                                h = 2 * hp + hh
                                qt = qrope[hp]
                                base = hh * 64
                                ps_g = psS.tile([128, 2, TQC], F32, tag="ps_g", name="ps_g")
                                los = []
                                for gi, ik in enumerate(iks):
                                    s = ik - 4 * jq
                                    lo = max(s, 0) * 128
                                    los.append(lo)
                                    nc.tensor.matmul(
                                        ps_g[:, gi, lo:TQC],
                                        lhsT=k2[base : base + 64, ik * 128 : (ik + 1) * 128],
                                        rhs=qt[base : base + 64, jq * TQC + lo : (jq + 1) * TQC],
                                        start=True,
                                        stop=True,
                                    )
                                ptile = ppool.tile([128, 2, TQC], BF16, tag="pt", name="ptile")
                                nc.scalar.activation(
                                    out=ptile[:], in_=ps_g[:], func=AF.Exp, scale=SCALE
                                )
                                for gi, ik in enumerate(iks):
                                    if ik - 4 * jq >= 0:
                                        lo = los[gi]
                                        nc.vector.tensor_mul(
                                            out=ptile[:, gi, lo : lo + 128],
                                            in0=ptile[:, gi, lo : lo + 128],
                                            in1=mk_sb[:, 0, 0:128],
                                        )
                                for gi, ik in enumerate(iks):
                                    lo = los[gi]
                                    nc.tensor.matmul(
                                        pys[hh][:, lo:TQC],
                                        lhsT=vhat[:, ik, :],
                                        rhs=ptile[:, gi, lo:TQC],
                                        start=(ik == 0),
                                        stop=(ik == nik - 1),
                                    )
                        # proj for the chunk this pass completed one step ago
                        if hp == 1 and jq > 0:
                            emit_proj(jq - 1)
                        for hh in range(2):
                            # single fast copy frees the PSUM accumulator; the
                            # normalize chain then runs off the critical path
                            ybuf = small.tile([65, TQC], F32, tag="ybuf", name="ybuf")
                            nc.vector.tensor_copy(out=ybuf[:], in_=pys[hh][:])
                            srow = small.tile([1, TQC], F32, tag="srow", name="srow")
                            nc.vector.tensor_copy(out=srow[:], in_=pys[hh][64:65, :])
                            rinv = small.tile([1, TQC], F32, tag="rinv", name="rinv")
                            nc.vector.reciprocal_approx_fast(out=rinv[:], in_=srow[:])
                            rb = small.tile([64, TQC], F32, tag="rb", name="rb")
                            nc.gpsimd.partition_broadcast(rb[:], rinv[:])
                            nc.vector.tensor_mul(
                                out=yn[hp][hh * 64 : hh * 64 + 64, jq * TQC : (jq + 1) * TQC],
                                in0=ybuf[0:64, :],
                                in1=rb[:],
                            )
                emit_proj(NJQ - 1)

    nc.compile()
    return nc


def _host_tables():
    # RoPE tables in interleaved-pair device layout (row j'=2i <-> orig j=i,
    # j'=2i+1 <-> orig j=i+32); sign of the shuffled sin term folded in.
    inv = 1.0 / (10000.0 ** (np.arange(0, D, 2, dtype=np.float64) / D))  # (32,)
    t = np.arange(T, dtype=np.float64)
    fr = np.outer(t, inv)  # (T, 32)
    cos_h = np.cos(fr).T.astype(np.float32)  # (32, T)
    sin_h = np.sin(fr).T.astype(np.float32)
    cosI = np.empty((D, T), np.float32)
    sinI = np.empty((D, T), np.float32)
    cosI[0::2] = cos_h
    cosI[1::2] = cos_h
    sinI[0::2] = -sin_h
    sinI[1::2] = sin_h
    cos_q = np.tile(cosI, (2, 1))
    sin_q = np.tile(sinI, (2, 1))
    # masks: staircase tile s of a tq-chunk: allowed iff s*128 + tkl <= tql
    tkl = np.arange(128)[:, None]
    tql = np.arange(TQC)[None, :]
    masks = np.stack(
        [(s * 128 + tkl <= tql).astype(np.float32) for s in range(4)], axis=1
    ).astype(ml_dtypes.bfloat16)  # (128, 4, TQC)
    identb = np.tile(np.eye(64, dtype=np.float32), (2, 2))
    return cos_q, sin_q, masks, identb


def make_in_maps(x, wq, wk, wv, wproj):
    cos_q, sin_q, masks, identb = _host_tables()
    # interleave permutation within each head's 64 cols: perm[2i]=i, perm[2i+1]=i+32
    perm = np.empty(D, np.int64)
    perm[0::2] = np.arange(32)
    perm[1::2] = np.arange(32) + 32
    in_maps = []
    for c in range(8):
        b, h = c // 4, c % 4
        xT = np.ascontiguousarray(x[b].T).astype(ml_dtypes.bfloat16)  # (C, T)
        wq_h = wq[:, h * 256 : (h + 1) * 256].reshape(C, G, D)[:, :, perm].reshape(C, 256)
        wk_h = wk[:, h * 64 : (h + 1) * 64][:, perm]
        wv_h = wv[:, h * 64 : (h + 1) * 64]
        w_all = np.concatenate([wq_h, wk_h, wv_h], axis=1).astype(ml_dtypes.bfloat16)
        wp_h = wproj[h * 256 : (h + 1) * 256, :].astype(ml_dtypes.bfloat16)
        in_maps.append(
            {
                "xT": xT,
                "w_all": w_all,
                "wp": wp_h,
                "cos_q": cos_q,
                "sin_q": sin_q,
                "masks": masks,
                "identb": identb,
            }
        )
    return in_maps


def kernel(x, wq, wk, wv, wproj):
    x = np.asarray(x, dtype=np.float32)
    wq = np.asarray(wq, dtype=np.float32)
    wk = np.asarray(wk, dtype=np.float32)
    wv = np.asarray(wv, dtype=np.float32)
    wproj = np.asarray(wproj, dtype=np.float32)
    B = x.shape[0]

    if "nc" not in _PROG:
        _PROG["nc"] = _build_program()
    nc = _PROG["nc"]

    in_maps = make_in_maps(x, wq, wk, wv, wproj)

    res = run_bass_kernel_spmd(nc, in_maps, list(range(8)))
    out = np.zeros((B, T, C), np.float32)
    for c in range(8):
        out[c // 4] += res.results[c]["out_p"]
    return out


# revision 13
# speedup vs baseline: 1.6387x; 1.0712x over previous
"""Causal self-attention (GQA + RoPE) Trainium2 kernel over 8 NeuronCores.

Sharding: 8 cores = batch(2) x kv_head(4). Each core computes its batch's
4 q-heads / 1 kv-head attention plus the partial output projection; host
sums the 4 partial projections per batch element.

Device algorithm (fully transposed "k-major" attention, zero P-transposes):
  stage A:  QKV^T = [wq|wk|wv]^T @ x^T on PE (bf16, fp32 accum); the Q1
            (heads 2,3) part is interleaved into attention pass 0 as PE
            filler so the tensor engine never idles (keeps HAM at 2.4GHz)
  RoPE on DVE via interleaved-pair layout (host permutes wq/wk columns so
    rotation partners are adjacent partitions -> stream_shuffle swap)
  V^T -> V via PE transposes; ones-column appended to V so the PV matmul
    emits both Y^T and the softmax sums in one accumulation
  attention in two head-pair passes (frees PSUM banks); S^T = K^T.T @ Q^T,
    one exp ACTIVATE per two S tiles (exp is elementwise; per-ik partition
    semantics only matter to the PV consumer), causal staircase trimming,
    single tri-mask multiply per diagonal block
  normalize: fast reciprocal + gpsimd partition_broadcast + DVE mul, off
    the critical path via an eager PSUM->SBUF copy
  projection: wproj rows for this core's heads (bf16), interleaved into
    pass 1; partial f32 out to HBM
"""

import sys

sys.path.insert(0, "/opt/trn_rl_repo")

import numpy as np
import ml_dtypes

import concourse.bacc as bacc
import concourse.tile as tile
from concourse import mybir
from concourse.bass_utils import run_bass_kernel_spmd

F32 = mybir.dt.float32
BF16 = mybir.dt.bfloat16
AF = mybir.ActivationFunctionType

T, C, D, H, HKV = 2048, 1024, 64, 16, 4
G = H // HKV  # q heads per kv head
NCC = C // 128  # 8 contraction chunks
NJQ = 4  # tq chunks of 512
TQC = 512
NTK = T // 128  # 16 tk tiles
SCALE = 1.0 / 8.0  # 1/sqrt(D)

_PROG = {}


def _build_program():
    nc = bacc.Bacc()
    xT_d = nc.dram_tensor("xT", [C, T], BF16, kind="ExternalInput")
    w_d = nc.dram_tensor("w_all", [C, 384], BF16, kind="ExternalInput")
    wp_d = nc.dram_tensor("wp", [256, C], BF16, kind="ExternalInput")
    cq_d = nc.dram_tensor("cos_q", [128, T], F32, kind="ExternalInput")
    sq_d = nc.dram_tensor("sin_q", [128, T], F32, kind="ExternalInput")
    mk_d = nc.dram_tensor("masks", [128, 128], BF16, kind="ExternalInput")
    id_d = nc.dram_tensor("identb", [128, 128], F32, kind="ExternalInput")
    out_d = nc.dram_tensor("out_p", [T, C], F32, kind="ExternalOutput")

    swap_mask = [i ^ 1 for i in range(32)]

    with tile.TileContext(nc) as tc:
        with (
            tc.tile_pool(name="const", bufs=1) as const,
            tc.tile_pool(name="big", bufs=1) as big,
            tc.tile_pool(name="xp", bufs=1) as xp,
            tc.tile_pool(name="rope", bufs=1) as ropep,
            tc.tile_pool(name="ptiles", bufs=6) as ppool,
            tc.tile_pool(name="small", bufs=3) as small,
            tc.tile_pool(name="outp", bufs=3) as outp,
        ):
            W_sb = const.tile([128, NCC, 384], BF16, tag="W", name="W_sb")
            cq_sb = const.tile([128, T], F32, tag="cq", name="cq_sb")
            sq_sb = const.tile([128, T], F32, tag="sq", name="sq_sb")
            mk_sb = const.tile([128, 128], BF16, tag="mk", name="mk_sb")
            id_sb = const.tile([128, 128], F32, tag="idb", name="id_sb")
            wp_sb = const.tile([128, 2, C], BF16, tag="wp", name="wp_sb")

            qkv_sb = [big.tile([128, T], F32, tag=f"qkv{m}", name=f"qkv{m}") for m in range(3)]
            qrope = [big.tile([128, T], BF16, tag=f"qr{m}", name=f"qr{m}") for m in range(2)]
            k2 = big.tile([128, T], BF16, tag="k2", name="k2")
            vhat = big.tile([128, NTK, 65], BF16, tag="vhat", name="vhat")
            yn = [big.tile([128, T], BF16, tag=f"yn{m}", name=f"yn{m}") for m in range(2)]

            xts = []
            for cc in range(NCC):
                xt = xp.tile([128, T], BF16, tag=f"x{cc}", name=f"x{cc}")
                nc.sync.dma_start(out=xt[:], in_=xT_d[cc * 128 : (cc + 1) * 128, :])
                nc.sync.dma_start(out=W_sb[:, cc, :], in_=w_d[cc * 128 : (cc + 1) * 128, :])
                xts.append(xt)
            nc.sync.dma_start(out=cq_sb[:], in_=cq_d[:])
            nc.sync.dma_start(out=sq_sb[:], in_=sq_d[:])
            nc.sync.dma_start(out=mk_sb[:], in_=mk_d[:])
            nc.sync.dma_start(out=id_sb[:], in_=id_d[:])
            nc.sync.dma_start(out=wp_sb[:], in_=wp_d[:].rearrange("(n p) m -> p n m", p=128))

            def emit_rope(pt):
                rows = 128 if pt < 2 else 64
                dst = qrope[pt] if pt < 2 else k2
                src = qkv_sb[pt]
                shuf = ropep.tile([128, T], F32, tag="shuf", name="shuf")
                prod = ropep.tile([128, T], F32, tag="prod", name="prod")
                nc.vector.stream_shuffle(shuf[:rows, :], src[:rows, :], mask=swap_mask)
                nc.vector.tensor_mul(out=shuf[:rows, :], in0=shuf[:rows, :], in1=sq_sb[:rows, :])
                nc.vector.tensor_mul(out=prod[:rows, :], in0=src[:rows, :], in1=cq_sb[:rows, :])
                nc.vector.tensor_add(out=dst[:rows, :], in0=prod[:rows, :], in1=shuf[:rows, :])

            # ---- stage A part 1: KV + Q0 projections, rope, Vhat ----
            with (
                tc.tile_pool(name="psA", bufs=1, space="PSUM") as psA,
                tc.tile_pool(name="psT", bufs=2, space="PSUM") as psT,
            ):
                for mt in (2, 0):
                    pas = [psA.tile([128, TQC], F32, tag=f"pa{j}", name=f"pa{j}") for j in range(NJQ)]
                    for cc in range(NCC):
                        for jq in range(NJQ):
                            nc.tensor.matmul(
                                pas[jq][:],
                                lhsT=W_sb[:, cc, mt * 128 : (mt + 1) * 128],
                                rhs=xts[cc][:, jq * TQC : (jq + 1) * TQC],
                                start=(cc == 0),
                                stop=(cc == NCC - 1),
                            )
                    for jq in range(NJQ):
                        nc.scalar.copy(
                            out=qkv_sb[mt][:, jq * TQC : (jq + 1) * TQC], in_=pas[jq][:]
                        )

                emit_rope(2)  # K first: attention depends on it
                # duplicate K^T into partitions 64:128 (head-pair row groups)
                nc.sync.dma_start(out=k2[64:128, :], in_=k2[0:64, :])
                emit_rope(0)

                # Vhat: V (t-major) + ones column for softmax sums
                nc.vector.memset(vhat[:, :, 64:65], 1.0)
                for tt in range(NTK):
                    pt_ = psT.tile([128, 64], F32, tag="ptr", name="ptr")
                    nc.tensor.transpose(
                        pt_[:],
                        qkv_sb[2][64:128, tt * 128 : (tt + 1) * 128],
                        id_sb[64:128, 0:64],
                    )
                    nc.vector.tensor_copy(out=vhat[:, tt, 0:64], in_=pt_[:])

            def attention_pass(hp, psS, psY, filler, boundary):
                """One head-pair pass. filler() emits one unit of extra PE work
                (stage-A Q1 / projection) per group to keep the PE dense;
                boundary(jq) runs after each chunk before the normalize."""
                for jq in range(NJQ):
                    nik = 4 * jq + 4
                    pys = [psY.tile([65, TQC], F32, tag="py", name="py") for _ in range(2)]
                    for ika in range(0, nik, 2):
                        iks = (ika, ika + 1)
                        filler()
                        for hh in range(2):
                            qt = qrope[hp]
                            base = hh * 64
                            ps_g = psS.tile([128, 2, TQC], F32, tag="ps_g", name="ps_g")
                            los = []
                            for gi, ik in enumerate(iks):
                                s = ik - 4 * jq
                                lo = max(s, 0) * 128
                                los.append(lo)
                                nc.tensor.matmul(
                                    ps_g[:, gi, lo:TQC],
                                    lhsT=k2[base : base + 64, ik * 128 : (ik + 1) * 128],
                                    rhs=qt[base : base + 64, jq * TQC + lo : (jq + 1) * TQC],
                                    start=True,
                                    stop=True,
                                )
                            ptile = ppool.tile([128, 2, TQC], BF16, tag="pt", name="ptile")
                            nc.scalar.activation(out=ptile[:], in_=ps_g[:], func=AF.Exp, scale=SCALE)
                            for gi, ik in enumerate(iks):
                                if ik - 4 * jq >= 0:
                                    lo = los[gi]
                                    nc.vector.tensor_mul(
                                        out=ptile[:, gi, lo : lo + 128],
                                        in0=ptile[:, gi, lo : lo + 128],
                                        in1=mk_sb[:, 0:128],
                                    )
                            for gi, ik in enumerate(iks):
                                lo = los[gi]
                                nc.tensor.matmul(
                                    pys[hh][:, lo:TQC],
                                    lhsT=vhat[:, ik, :],
                                    rhs=ptile[:, gi, lo:TQC],
                                    start=(ik == 0),
                                    stop=(ik == nik - 1),
                                )
                    boundary(jq)
                    for hh in range(2):
                        # eager copy frees the PSUM accumulator; normalize runs
                        # off the critical path
                        ybuf = small.tile([65, TQC], F32, tag="ybuf", name="ybuf")
                        nc.vector.tensor_copy(out=ybuf[:], in_=pys[hh][:])
                        srow = small.tile([1, TQC], F32, tag="srow", name="srow")
                        nc.vector.tensor_copy(out=srow[:], in_=ybuf[64:65, :])
                        rinv = small.tile([1, TQC], F32, tag="rinv", name="rinv")
                        nc.vector.reciprocal_approx_fast(out=rinv[:], in_=srow[:])
                        rb = small.tile([64, TQC], F32, tag="rb", name="rb")
                        nc.gpsimd.partition_broadcast(rb[:], rinv[:])
                        nc.vector.tensor_mul(
                            out=yn[hp][hh * 64 : hh * 64 + 64, jq * TQC : (jq + 1) * TQC],
                            in0=ybuf[0:64, :],
                            in1=rb[:],
                        )

            # ---- pass 0 (heads 0,1) with stage-A Q1 interleaved ----
            with (
                tc.tile_pool(name="psS0", bufs=2, space="PSUM") as psS0,
                tc.tile_pool(name="psY0", bufs=2, space="PSUM") as psY0,
                tc.tile_pool(name="psA2", bufs=2, space="PSUM") as psA2,
            ):

                def a2_gen():
                    for jq2 in range(NJQ):
                        pa = psA2.tile([128, TQC], F32, tag="pa2", name="pa2")
                        for cc in range(NCC):
                            nc.tensor.matmul(
                                pa[:],
                                lhsT=W_sb[:, cc, 128:256],
                                rhs=xts[cc][:, jq2 * TQC : (jq2 + 1) * TQC],
                                start=(cc == 0),
                                stop=(cc == NCC - 1),
                            )
                            yield
                        nc.scalar.copy(
                            out=qkv_sb[1][:, jq2 * TQC : (jq2 + 1) * TQC], in_=pa[:]
                        )
                    emit_rope(1)
                    yield

                gen = a2_gen()

                def filler0():
                    next(gen, None)

                # head start for the PE while rope-q0 finishes on DVE
                for _ in range(4):
                    filler0()
                attention_pass(0, psS0, psY0, filler0, lambda jq: None)
                for _ in gen:
                    pass

            # ---- pass 1 (heads 2,3) with projection interleaved ----
            with (
                tc.tile_pool(name="psS1", bufs=2, space="PSUM") as psS1,
                tc.tile_pool(name="psY1", bufs=2, space="PSUM") as psY1,
                tc.tile_pool(name="psP", bufs=1, space="PSUM") as psP,
            ):

                def emit_proj(pjq):
                    for tt in range(4 * pjq, 4 * pjq + 4):
                        outsb = outp.tile([128, C], F32, tag="osb", name="osb")
                        pps = [psP.tile([128, 512], F32, tag=f"pp{n}", name=f"pp{n}") for n in range(2)]
                        for kk in range(2):
                            for ncol in range(2):
                                nc.tensor.matmul(
                                    pps[ncol][:],
                                    lhsT=yn[kk][:, tt * 128 : (tt + 1) * 128],
                                    rhs=wp_sb[:, kk, ncol * 512 : (ncol + 1) * 512],
                                    start=(kk == 0),
                                    stop=(kk == 1),
                                )
                        for ncol in range(2):
                            nc.vector.tensor_copy(
                                out=outsb[:, ncol * 512 : (ncol + 1) * 512], in_=pps[ncol][:]
                            )
                        nc.sync.dma_start(
                            out=out_d[tt * 128 : (tt + 1) * 128, :], in_=outsb[:]
                        )

                def boundary1(jq):
                    if jq > 0:
                        emit_proj(jq - 1)

                attention_pass(1, psS1, psY1, lambda: None, boundary1)
                emit_proj(NJQ - 1)

    nc.compile()
    return nc


def _host_tables():
    # RoPE tables in interleaved-pair device layout (row j'=2i <-> orig j=i,
    # j'=2i+1 <-> orig j=i+32); sign of the shuffled sin term folded in.
    inv = 1.0 / (10000.0 ** (np.arange(0, D, 2, dtype=np.float64) / D))  # (32,)
    t = np.arange(T, dtype=np.float64)
    fr = np.outer(t, inv)  # (T, 32)
    cos_h = np.cos(fr).T.astype(np.float32)  # (32, T)
    sin_h = np.sin(fr).T.astype(np.float32)
    cosI = np.empty((D, T), np.float32)
    sinI = np.empty((D, T), np.float32)
    cosI[0::2] = cos_h
    cosI[1::2] = cos_h
    sinI[0::2] = -sin_h
    sinI[1::2] = sin_h
    cos_q = np.tile(cosI, (2, 1))
    sin_q = np.tile(sinI, (2, 1))
    # tri mask for the diagonal 128-block: allowed iff tkl <= tql
    tkl = np.arange(128)[:, None]
    tql = np.arange(128)[None, :]
    mask = (tkl <= tql).astype(np.float32).astype(ml_dtypes.bfloat16)
    identb = np.tile(np.eye(64, dtype=np.float32), (2, 2))
    return cos_q, sin_q, mask, identb


def make_in_maps(x, wq, wk, wv, wproj):
    cos_q, sin_q, mask, identb = _host_tables()
    # interleave permutation within each head's 64 cols: perm[2i]=i, perm[2i+1]=i+32
    perm = np.empty(D, np.int64)
    perm[0::2] = np.arange(32)
    perm[1::2] = np.arange(32) + 32
    in_maps = []
    for c in range(8):
        b, h = c // 4, c % 4
        xT = np.ascontiguousarray(x[b].T).astype(ml_dtypes.bfloat16)  # (C, T)
        wq_h = wq[:, h * 256 : (h + 1) * 256].reshape(C, G, D)[:, :, perm].reshape(C, 256)
        wk_h = wk[:, h * 64 : (h + 1) * 64][:, perm]
        wv_h = wv[:, h * 64 : (h + 1) * 64]
        w_all = np.concatenate([wq_h, wk_h, wv_h], axis=1).astype(ml_dtypes.bfloat16)
        wp_h = wproj[h * 256 : (h + 1) * 256, :].astype(ml_dtypes.bfloat16)
        in_maps.append(
            {
                "xT": xT,
                "w_all": w_all,
                "wp": wp_h,
                "cos_q": cos_q,
                "sin_q": sin_q,
                "masks": mask,
                "identb": identb,
            }
        )
    return in_maps


def kernel(x, wq, wk, wv, wproj):
    x = np.asarray(x, dtype=np.float32)
    wq = np.asarray(wq, dtype=np.float32)
    wk = np.asarray(wk, dtype=np.float32)
    wv = np.asarray(wv, dtype=np.float32)
    wproj = np.asarray(wproj, dtype=np.float32)
    B = x.shape[0]

    if "nc" not in _PROG:
        _PROG["nc"] = _build_program()
    nc = _PROG["nc"]

    in_maps = make_in_maps(x, wq, wk, wv, wproj)

    res = run_bass_kernel_spmd(nc, in_maps, list(range(8)))
    out = np.zeros((B, T, C), np.float32)
    for c in range(8):
        out[c // 4] += res.results[c]["out_p"]
    return out


# revision 16
# speedup vs baseline: 1.6795x; 1.0249x over previous
"""Causal self-attention (GQA + RoPE) Trainium2 kernel over 8 NeuronCores.

Sharding: 8 cores = batch(2) x kv_head(4). Each core computes its batch's
4 q-heads / 1 kv-head attention plus the partial output projection; host
sums the 4 partial projections per batch element.

Device algorithm (fully transposed "k-major" attention, zero P-transposes):
  stage A:  QKV^T = [wq|wk|wv]^T @ x^T on PE (bf16, fp32 accum); the Q1
            (heads 2,3) part is interleaved into attention pass 0 as PE
            filler so the tensor engine never idles (keeps HAM at 2.4GHz)
  RoPE on DVE via interleaved-pair layout (host permutes wq/wk columns so
    rotation partners are adjacent partitions -> stream_shuffle swap)
  V^T -> V via PE transposes; ones-column appended to V so the PV matmul
    emits both Y^T and the softmax sums in one accumulation
  attention in two head-pair passes (frees PSUM banks); S^T = K^T.T @ Q^T,
    one exp ACTIVATE per two S tiles (exp is elementwise; per-ik partition
    semantics only matter to the PV consumer), causal staircase trimming,
    single tri-mask multiply per diagonal block
  normalize: fast reciprocal + gpsimd partition_broadcast + DVE mul, off
    the critical path via an eager PSUM->SBUF copy
  projection: wproj rows for this core's heads (bf16), interleaved into
    pass 1; partial f32 out to HBM
"""

import sys

sys.path.insert(0, "/opt/trn_rl_repo")

import numpy as np
import ml_dtypes

import concourse.bacc as bacc
import concourse.tile as tile
from concourse import mybir
from concourse.bass_utils import run_bass_kernel_spmd

F32 = mybir.dt.float32
BF16 = mybir.dt.bfloat16
AF = mybir.ActivationFunctionType

T, C, D, H, HKV = 2048, 1024, 64, 16, 4
G = H // HKV  # q heads per kv head
NCC = C // 128  # 8 contraction chunks
NJQ = 4  # tq chunks of 512
TQC = 512
NTK = T // 128  # 16 tk tiles
SCALE = 1.0 / 8.0  # 1/sqrt(D)

_PROG = {}


def _build_program():
    nc = bacc.Bacc()
    xT_d = nc.dram_tensor("xT", [C, T], BF16, kind="ExternalInput")
    w_d = nc.dram_tensor("w_all", [C, 384], BF16, kind="ExternalInput")
    wp_d = nc.dram_tensor("wp", [256, C], BF16, kind="ExternalInput")
    cq_d = nc.dram_tensor("cos_q", [128, T], F32, kind="ExternalInput")
    sq_d = nc.dram_tensor("sin_q", [128, T], F32, kind="ExternalInput")
    mk_d = nc.dram_tensor("masks", [128, 128], BF16, kind="ExternalInput")
    id_d = nc.dram_tensor("identb", [128, 128], F32, kind="ExternalInput")
    out_d = nc.dram_tensor("out_p", [T, C], F32, kind="ExternalOutput")

    swap_mask = [i ^ 1 for i in range(32)]

    with tile.TileContext(nc) as tc:
        with (
            tc.tile_pool(name="const", bufs=1) as const,
            tc.tile_pool(name="big", bufs=1) as big,
            tc.tile_pool(name="xp", bufs=1) as xp,
            tc.tile_pool(name="rope", bufs=1) as ropep,
            tc.tile_pool(name="ptiles", bufs=8) as ppool,
            tc.tile_pool(name="small", bufs=3) as small,
            tc.tile_pool(name="outp", bufs=3) as outp,
        ):
            W_sb = const.tile([128, NCC, 384], BF16, tag="W", name="W_sb")
            cq_sb = const.tile([128, T], F32, tag="cq", name="cq_sb")
            sq_sb = const.tile([128, T], F32, tag="sq", name="sq_sb")
            mk_sb = const.tile([128, 128], BF16, tag="mk", name="mk_sb")
            id_sb = const.tile([128, 128], F32, tag="idb", name="id_sb")
            wp_sb = const.tile([128, 2, C], BF16, tag="wp", name="wp_sb")

            qkv_sb = [big.tile([128, T], F32, tag=f"qkv{m}", name=f"qkv{m}") for m in range(3)]
            qrope = [big.tile([128, T], BF16, tag=f"qr{m}", name=f"qr{m}") for m in range(2)]
            k2 = big.tile([128, T], BF16, tag="k2", name="k2")
            vhat = big.tile([128, NTK, 65], BF16, tag="vhat", name="vhat")
            yn = [big.tile([128, T], BF16, tag=f"yn{m}", name=f"yn{m}") for m in range(2)]

            xts = []
            for cc in range(NCC):
                xt = xp.tile([128, T], BF16, tag=f"x{cc}", name=f"x{cc}")
                nc.sync.dma_start(out=xt[:], in_=xT_d[cc * 128 : (cc + 1) * 128, :])
                nc.sync.dma_start(out=W_sb[:, cc, :], in_=w_d[cc * 128 : (cc + 1) * 128, :])
                xts.append(xt)
            nc.sync.dma_start(out=cq_sb[:], in_=cq_d[:])
            nc.sync.dma_start(out=sq_sb[:], in_=sq_d[:])
            nc.sync.dma_start(out=mk_sb[:], in_=mk_d[:])
            nc.sync.dma_start(out=id_sb[:], in_=id_d[:])
            nc.sync.dma_start(out=wp_sb[:], in_=wp_d[:].rearrange("(n p) m -> p n m", p=128))

            def emit_rope(pt, jqs=None):
                rows = 128 if pt < 2 else 64
                dst = qrope[pt] if pt < 2 else k2
                src = qkv_sb[pt]
                for j in jqs if jqs is not None else range(NJQ):
                    cs = slice(j * TQC, (j + 1) * TQC)
                    shuf = ropep.tile([128, TQC], F32, tag="shuf", name="shuf")
                    prod = ropep.tile([128, TQC], F32, tag="prod", name="prod")
                    nc.vector.stream_shuffle(shuf[:rows, :], src[:rows, cs], mask=swap_mask)
                    nc.vector.tensor_mul(out=shuf[:rows, :], in0=shuf[:rows, :], in1=sq_sb[:rows, cs])
                    nc.vector.tensor_mul(out=prod[:rows, :], in0=src[:rows, cs], in1=cq_sb[:rows, cs])
                    nc.vector.tensor_add(out=dst[:rows, cs], in0=prod[:rows, :], in1=shuf[:rows, :])

            # ---- stage A part 1: KV + Q0 projections, rope, Vhat ----
            with (
                tc.tile_pool(name="psA", bufs=1, space="PSUM") as psA,
                tc.tile_pool(name="psT", bufs=2, space="PSUM") as psT,
            ):
                for mt in (2, 0):
                    pas = [psA.tile([128, TQC], F32, tag=f"pa{j}", name=f"pa{j}") for j in range(NJQ)]
                    for cc in range(NCC):
                        for jq in range(NJQ):
                            nc.tensor.matmul(
                                pas[jq][:],
                                lhsT=W_sb[:, cc, mt * 128 : (mt + 1) * 128],
                                rhs=xts[cc][:, jq * TQC : (jq + 1) * TQC],
                                start=(cc == 0),
                                stop=(cc == NCC - 1),
                            )
                    for jq in range(NJQ):
                        nc.scalar.copy(
                            out=qkv_sb[mt][:, jq * TQC : (jq + 1) * TQC], in_=pas[jq][:]
                        )

                emit_rope(2)  # K first: attention depends on it
                # duplicate K^T into partitions 64:128 (head-pair row groups)
                nc.sync.dma_start(out=k2[64:128, :], in_=k2[0:64, :])
                emit_rope(0, jqs=(3, 2, 1, 0))

                # Vhat: V (t-major) + ones column for softmax sums
                nc.vector.memset(vhat[:, :, 64:65], 1.0)
                for tt in range(NTK):
                    pt_ = psT.tile([128, 64], F32, tag="ptr", name="ptr")
                    nc.tensor.transpose(
                        pt_[:],
                        qkv_sb[2][64:128, tt * 128 : (tt + 1) * 128],
                        id_sb[64:128, 0:64],
                    )
                    nc.vector.tensor_copy(out=vhat[:, tt, 0:64], in_=pt_[:])

            def attention_pass(hp, psS, psY, filler, boundary, jq_order=tuple(range(NJQ))):
                """One head-pair pass. filler() emits one unit of extra PE work
                (stage-A Q1 / projection) per group to keep the PE dense;
                boundary(jq) runs after each chunk before the normalize."""
                for jq in jq_order:
                    nik = 4 * jq + 4
                    pys = [psY.tile([65, TQC], F32, tag="py", name="py") for _ in range(2)]
                    for ika in range(0, nik, 2):
                        iks = (ika, ika + 1)
                        filler()
                        qt = qrope[hp]
                        los = []
                        for gi, ik in enumerate(iks):
                            s = ik - 4 * jq
                            los.append(max(s, 0) * 128)
                        ps_gs = [
                            psS.tile([128, 2, TQC], F32, tag=f"ps_g{hh}", name=f"ps_g{hh}")
                            for hh in range(2)
                        ]
                        # alternate row groups (hh base 0 / 64) so consecutive
                        # half-array S matmuls overlap in the PE array
                        for gi, ik in enumerate(iks):
                            lo = los[gi]
                            for hh in range(2):
                                base = hh * 64
                                nc.tensor.matmul(
                                    ps_gs[hh][:, gi, lo:TQC],
                                    lhsT=k2[base : base + 64, ik * 128 : (ik + 1) * 128],
                                    rhs=qt[base : base + 64, jq * TQC + lo : (jq + 1) * TQC],
                                    start=True,
                                    stop=True,
                                )
                        ptiles = []
                        for hh in range(2):
                            ptile = ppool.tile([128, 2, TQC], BF16, tag="pt", name="ptile")
                            nc.scalar.activation(
                                out=ptile[:], in_=ps_gs[hh][:], func=AF.Exp, scale=SCALE
                            )
                            ptiles.append(ptile)
                        for hh in range(2):
                            for gi, ik in enumerate(iks):
                                if ik - 4 * jq >= 0:
                                    lo = los[gi]
                                    nc.vector.tensor_mul(
                                        out=ptiles[hh][:, gi, lo : lo + 128],
                                        in0=ptiles[hh][:, gi, lo : lo + 128],
                                        in1=mk_sb[:, 0:128],
                                    )
                        for hh in range(2):
                            for gi, ik in enumerate(iks):
                                lo = los[gi]
                                nc.tensor.matmul(
                                    pys[hh][:, lo:TQC],
                                    lhsT=vhat[:, ik, :],
                                    rhs=ptiles[hh][:, gi, lo:TQC],
                                    start=(ik == 0),
                                    stop=(ik == nik - 1),
                                )
                    boundary(jq)
                    for hh in range(2):
                        # eager copy frees the PSUM accumulator; normalize runs
                        # off the critical path
                        ybuf = small.tile([65, TQC], F32, tag="ybuf", name="ybuf")
                        nc.vector.tensor_copy(out=ybuf[:], in_=pys[hh][:])
                        srow = small.tile([1, TQC], F32, tag="srow", name="srow")
                        nc.vector.tensor_copy(out=srow[:], in_=pys[hh][64:65, :])
                        rinv = small.tile([1, TQC], F32, tag="rinv", name="rinv")
                        nc.vector.reciprocal_approx_fast(out=rinv[:], in_=srow[:])
                        rb = small.tile([64, TQC], F32, tag="rb", name="rb")
                        nc.gpsimd.partition_broadcast(rb[:], rinv[:])
                        nc.vector.tensor_mul(
                            out=yn[hp][hh * 64 : hh * 64 + 64, jq * TQC : (jq + 1) * TQC],
                            in0=ybuf[0:64, :],
                            in1=rb[:],
                        )

            # ---- pass 0 (heads 0,1) with stage-A Q1 interleaved ----
            with (
                tc.tile_pool(name="psS0", bufs=1, space="PSUM") as psS0,
                tc.tile_pool(name="psY0", bufs=2, space="PSUM") as psY0,
                tc.tile_pool(name="psA2", bufs=2, space="PSUM") as psA2,
            ):

                def a2_gen():
                    for jq2 in range(NJQ):
                        pa = psA2.tile([128, TQC], F32, tag="pa2", name="pa2")
                        for cc in range(NCC):
                            nc.tensor.matmul(
                                pa[:],
                                lhsT=W_sb[:, cc, 128:256],
                                rhs=xts[cc][:, jq2 * TQC : (jq2 + 1) * TQC],
                                start=(cc == 0),
                                stop=(cc == NCC - 1),
                            )
                            yield
                        nc.scalar.copy(
                            out=qkv_sb[1][:, jq2 * TQC : (jq2 + 1) * TQC], in_=pa[:]
                        )
                    emit_rope(1)
                    yield

                gen = a2_gen()

                def filler0():
                    next(gen, None)

                def filler0x2():
                    filler0()
                    filler0()

                # head start for the PE while rope-q0 finishes on DVE
                for _ in range(4):
                    filler0()
                attention_pass(
                    0, psS0, psY0, filler0x2, lambda jq: None, jq_order=(3, 2, 1, 0)
                )
                for _ in gen:
                    pass

            # ---- pass 1 (heads 2,3) with projection interleaved ----
            with (
                tc.tile_pool(name="psS1", bufs=1, space="PSUM") as psS1,
                tc.tile_pool(name="psY1", bufs=2, space="PSUM") as psY1,
                tc.tile_pool(name="psP", bufs=1, space="PSUM") as psP,
            ):

                def emit_proj(pjq):
                    for tt in range(4 * pjq, 4 * pjq + 4):
                        outsb = outp.tile([128, C], F32, tag="osb", name="osb")
                        pps = [psP.tile([128, 512], F32, tag=f"pp{n}", name=f"pp{n}") for n in range(2)]
                        for kk in range(2):
                            for ncol in range(2):
                                nc.tensor.matmul(
                                    pps[ncol][:],
                                    lhsT=yn[kk][:, tt * 128 : (tt + 1) * 128],
                                    rhs=wp_sb[:, kk, ncol * 512 : (ncol + 1) * 512],
                                    start=(kk == 0),
                                    stop=(kk == 1),
                                )
                        for ncol in range(2):
                            nc.vector.tensor_copy(
                                out=outsb[:, ncol * 512 : (ncol + 1) * 512], in_=pps[ncol][:]
                            )
                        nc.sync.dma_start(
                            out=out_d[tt * 128 : (tt + 1) * 128, :], in_=outsb[:]
                        )

                def boundary1(jq):
                    if jq > 0:
                        emit_proj(jq - 1)

                attention_pass(1, psS1, psY1, lambda: None, boundary1)
                emit_proj(NJQ - 1)

    nc.compile()
    return nc


def _host_tables():
    # RoPE tables in interleaved-pair device layout (row j'=2i <-> orig j=i,
    # j'=2i+1 <-> orig j=i+32); sign of the shuffled sin term folded in.
    inv = 1.0 / (10000.0 ** (np.arange(0, D, 2, dtype=np.float64) / D))  # (32,)
    t = np.arange(T, dtype=np.float64)
    fr = np.outer(t, inv)  # (T, 32)
    cos_h = np.cos(fr).T.astype(np.float32)  # (32, T)
    sin_h = np.sin(fr).T.astype(np.float32)
    cosI = np.empty((D, T), np.float32)
    sinI = np.empty((D, T), np.float32)
    cosI[0::2] = cos_h
    cosI[1::2] = cos_h
    sinI[0::2] = -sin_h
    sinI[1::2] = sin_h
    cos_q = np.tile(cosI, (2, 1))
    sin_q = np.tile(sinI, (2, 1))
    # tri mask for the diagonal 128-block: allowed iff tkl <= tql
    tkl = np.arange(128)[:, None]
    tql = np.arange(128)[None, :]
    mask = (tkl <= tql).astype(np.float32).astype(ml_dtypes.bfloat16)
    identb = np.tile(np.eye(64, dtype=np.float32), (2, 2))
    return cos_q, sin_q, mask, identb


def make_in_maps(x, wq, wk, wv, wproj):
    cos_q, sin_q, mask, identb = _host_tables()
    # interleave permutation within each head's 64 cols: perm[2i]=i, perm[2i+1]=i+32
    perm = np.empty(D, np.int64)
    perm[0::2] = np.arange(32)
    perm[1::2] = np.arange(32) + 32
    in_maps = []
    for c in range(8):
        b, h = c // 4, c % 4
        xT = np.ascontiguousarray(x[b].T).astype(ml_dtypes.bfloat16)  # (C, T)
        wq_h = wq[:, h * 256 : (h + 1) * 256].reshape(C, G, D)[:, :, perm].reshape(C, 256)
        wk_h = wk[:, h * 64 : (h + 1) * 64][:, perm]
        wv_h = wv[:, h * 64 : (h + 1) * 64]
        w_all = np.concatenate([wq_h, wk_h, wv_h], axis=1).astype(ml_dtypes.bfloat16)
        wp_h = wproj[h * 256 : (h + 1) * 256, :].astype(ml_dtypes.bfloat16)
        in_maps.append(
            {
                "xT": xT,
                "w_all": w_all,
                "wp": wp_h,
                "cos_q": cos_q,
                "sin_q": sin_q,
                "masks": mask,
                "identb": identb,
            }
        )
    return in_maps


def kernel(x, wq, wk, wv, wproj):
    x = np.asarray(x, dtype=np.float32)
    wq = np.asarray(wq, dtype=np.float32)
    wk = np.asarray(wk, dtype=np.float32)
    wv = np.asarray(wv, dtype=np.float32)
    wproj = np.asarray(wproj, dtype=np.float32)
    B = x.shape[0]

    if "nc" not in _PROG:
        _PROG["nc"] = _build_program()
    nc = _PROG["nc"]

    in_maps = make_in_maps(x, wq, wk, wv, wproj)

    res = run_bass_kernel_spmd(nc, in_maps, list(range(8)))
    out = np.zeros((B, T, C), np.float32)
    for c in range(8):
        out[c // 4] += res.results[c]["out_p"]
    return out


# revision 18
# speedup vs baseline: 1.7122x; 1.0195x over previous
"""Causal self-attention (GQA + RoPE) Trainium2 kernel over 8 NeuronCores.

Sharding: 8 cores = batch(2) x kv_head(4). Each core computes its batch's
4 q-heads / 1 kv-head attention plus the partial output projection; host
sums the 4 partial projections per batch element.

Device algorithm (fully transposed "k-major" attention, zero P-transposes):
  stage A:  QKV^T = [wq|wk|wv]^T @ x^T on PE (bf16, fp32 accum); the Q1
            (heads 2,3) part is interleaved into attention pass 0 as PE
            filler so the tensor engine never idles (keeps HAM at 2.4GHz)
  RoPE on DVE via interleaved-pair layout (host permutes wq/wk columns so
    rotation partners are adjacent partitions -> stream_shuffle swap)
  V^T -> V via PE transposes; ones-column appended to V so the PV matmul
    emits both Y^T and the softmax sums in one accumulation
  attention in two head-pair passes (frees PSUM banks); S^T = K^T.T @ Q^T,
    one exp ACTIVATE per two S tiles (exp is elementwise; per-ik partition
    semantics only matter to the PV consumer), causal staircase trimming,
    single tri-mask multiply per diagonal block
  normalize: fast reciprocal + gpsimd partition_broadcast + DVE mul, off
    the critical path via an eager PSUM->SBUF copy
  projection: wproj rows for this core's heads (bf16), interleaved into
    pass 1; partial f32 out to HBM
"""

import sys

sys.path.insert(0, "/opt/trn_rl_repo")

import numpy as np
import ml_dtypes

import concourse.bacc as bacc
import concourse.tile as tile
from concourse import mybir
from concourse.bass_utils import run_bass_kernel_spmd

F32 = mybir.dt.float32
BF16 = mybir.dt.bfloat16
AF = mybir.ActivationFunctionType

T, C, D, H, HKV = 2048, 1024, 64, 16, 4
G = H // HKV  # q heads per kv head
NCC = C // 128  # 8 contraction chunks
NJQ = 4  # tq chunks of 512
TQC = 512
NTK = T // 128  # 16 tk tiles
SCALE = 1.0 / 8.0  # 1/sqrt(D)

_PROG = {}


def _build_program():
    nc = bacc.Bacc()
    xT_d = nc.dram_tensor("xT", [C, T], BF16, kind="ExternalInput")
    w_d = nc.dram_tensor("w_all", [C, 384], BF16, kind="ExternalInput")
    wp_d = nc.dram_tensor("wp", [256, C], BF16, kind="ExternalInput")
    cq_d = nc.dram_tensor("cos_q", [128, T], F32, kind="ExternalInput")
    sq_d = nc.dram_tensor("sin_q", [128, T], F32, kind="ExternalInput")
    mk_d = nc.dram_tensor("masks", [128, 128], BF16, kind="ExternalInput")
    id_d = nc.dram_tensor("identb", [128, 128], F32, kind="ExternalInput")
    out_d = nc.dram_tensor("out_p", [T, C], F32, kind="ExternalOutput")

    swap_mask = [i ^ 1 for i in range(32)]

    with tile.TileContext(nc) as tc:
        with (
            tc.tile_pool(name="const", bufs=1) as const,
            tc.tile_pool(name="big", bufs=1) as big,
            tc.tile_pool(name="xp", bufs=1) as xp,
            tc.tile_pool(name="rope", bufs=1) as ropep,
            tc.tile_pool(name="ptiles", bufs=8) as ppool,
            tc.tile_pool(name="small", bufs=3) as small,
            tc.tile_pool(name="outp", bufs=3) as outp,
        ):
            W_sb = const.tile([128, NCC, 384], BF16, tag="W", name="W_sb")
            cq_sb = const.tile([128, T], F32, tag="cq", name="cq_sb")
            sq_sb = const.tile([128, T], F32, tag="sq", name="sq_sb")
            mk_sb = const.tile([128, 128], BF16, tag="mk", name="mk_sb")
            id_sb = const.tile([128, 128], F32, tag="idb", name="id_sb")
            wp_sb = const.tile([128, 2, C], BF16, tag="wp", name="wp_sb")

            qkv_sb = [big.tile([128, T], F32, tag=f"qkv{m}", name=f"qkv{m}") for m in range(3)]
            qrope = [big.tile([128, T], BF16, tag=f"qr{m}", name=f"qr{m}") for m in range(2)]
            k2 = big.tile([128, T], BF16, tag="k2", name="k2")
            vhat = big.tile([128, NTK, 65], BF16, tag="vhat", name="vhat")
            yn = [big.tile([128, T], BF16, tag=f"yn{m}", name=f"yn{m}") for m in range(2)]

            xts = []
            for cc in range(NCC):
                xt = xp.tile([128, T], BF16, tag=f"x{cc}", name=f"x{cc}")
                nc.sync.dma_start(out=xt[:], in_=xT_d[cc * 128 : (cc + 1) * 128, :])
                nc.sync.dma_start(out=W_sb[:, cc, :], in_=w_d[cc * 128 : (cc + 1) * 128, :])
                xts.append(xt)
            nc.sync.dma_start(out=cq_sb[:], in_=cq_d[:])
            nc.sync.dma_start(out=sq_sb[:], in_=sq_d[:])
            nc.sync.dma_start(out=mk_sb[:], in_=mk_d[:])
            nc.sync.dma_start(out=id_sb[:], in_=id_d[:])
            nc.sync.dma_start(out=wp_sb[:], in_=wp_d[:].rearrange("(n p) m -> p n m", p=128))

            def emit_rope(pt, jqs=None):
                rows = 128 if pt < 2 else 64
                dst = qrope[pt] if pt < 2 else k2
                src = qkv_sb[pt]
                for j in jqs if jqs is not None else range(NJQ):
                    cs = slice(j * TQC, (j + 1) * TQC)
                    shuf = ropep.tile([128, TQC], F32, tag="shuf", name="shuf")
                    prod = ropep.tile([128, TQC], F32, tag="prod", name="prod")
                    nc.vector.stream_shuffle(shuf[:rows, :], src[:rows, cs], mask=swap_mask)
                    nc.vector.tensor_mul(out=shuf[:rows, :], in0=shuf[:rows, :], in1=sq_sb[:rows, cs])
                    nc.vector.tensor_mul(out=prod[:rows, :], in0=src[:rows, cs], in1=cq_sb[:rows, cs])
                    nc.vector.tensor_add(out=dst[:rows, cs], in0=prod[:rows, :], in1=shuf[:rows, :])

            # ---- stage A part 1: KV + Q0 projections, rope, Vhat ----
            with (
                tc.tile_pool(name="psA", bufs=1, space="PSUM") as psA,
                tc.tile_pool(name="psT", bufs=2, space="PSUM") as psT,
            ):
                # warmup: garbage matmuls with no input deps keep the PE busy
                # through the initial DMA window so HAM reaches 2.4GHz before
                # stage A starts (values never read; psum overwritten later)
                wu = ropep.tile([128, 512], BF16, tag="wu", name="wu")
                nc.vector.memset(wu[:], 0.5)
                puw = psT.tile([128, 512], F32, tag="ptr", name="puw")
                for _ in range(24):
                    nc.tensor.matmul(puw[:], lhsT=wu[:, 0:128], rhs=wu[:], start=True, stop=True)
                for mt in (2, 0):
                    pas = [psA.tile([128, TQC], F32, tag=f"pa{j}", name=f"pa{j}") for j in range(NJQ)]
                    for cc in range(NCC):
                        for jq in range(NJQ):
                            nc.tensor.matmul(
                                pas[jq][:],
                                lhsT=W_sb[:, cc, mt * 128 : (mt + 1) * 128],
                                rhs=xts[cc][:, jq * TQC : (jq + 1) * TQC],
                                start=(cc == 0),
                                stop=(cc == NCC - 1),
                            )
                    for jq in range(NJQ):
                        nc.scalar.copy(
                            out=qkv_sb[mt][:, jq * TQC : (jq + 1) * TQC], in_=pas[jq][:]
                        )

                emit_rope(2)  # K first: attention depends on it
                # duplicate K^T into partitions 64:128 (head-pair row groups)
                nc.sync.dma_start(out=k2[64:128, :], in_=k2[0:64, :])
                emit_rope(0, jqs=(3, 2, 1, 0))

                # Vhat: V (t-major) + ones column for softmax sums
                nc.vector.memset(vhat[:, :, 64:65], 1.0)
                for tt in range(NTK):
                    pt_ = psT.tile([128, 64], F32, tag="ptr", name="ptr")
                    nc.tensor.transpose(
                        pt_[:],
                        qkv_sb[2][64:128, tt * 128 : (tt + 1) * 128],
                        id_sb[64:128, 0:64],
                    )
                    nc.vector.tensor_copy(out=vhat[:, tt, 0:64], in_=pt_[:])

            def attention_pass(hp, psS, psY, filler, boundary, jq_order=tuple(range(NJQ))):
                """One head-pair pass. filler() emits one unit of extra PE work
                (stage-A Q1 / projection) per group to keep the PE dense;
                boundary(jq) runs after each chunk before the normalize."""
                for jq in jq_order:
                    nik = 4 * jq + 4
                    pys = [psY.tile([65, TQC], F32, tag="py", name="py") for _ in range(2)]
                    for ika in range(0, nik, 2):
                        iks = (ika, ika + 1)
                        filler()
                        qt = qrope[hp]
                        los = []
                        for gi, ik in enumerate(iks):
                            s = ik - 4 * jq
                            los.append(max(s, 0) * 128)
                        ps_gs = [
                            psS.tile([128, 2, TQC], F32, tag=f"ps_g{hh}", name=f"ps_g{hh}")
                            for hh in range(2)
                        ]
                        # alternate row groups (hh base 0 / 64) so consecutive
                        # half-array S matmuls overlap in the PE array
                        for gi, ik in enumerate(iks):
                            lo = los[gi]
                            for hh in range(2):
                                base = hh * 64
                                nc.tensor.matmul(
                                    ps_gs[hh][:, gi, lo:TQC],
                                    lhsT=k2[base : base + 64, ik * 128 : (ik + 1) * 128],
                                    rhs=qt[base : base + 64, jq * TQC + lo : (jq + 1) * TQC],
                                    start=True,
                                    stop=True,
                                )
                        mlo = min(los)
                        ptiles = []
                        for hh in range(2):
                            ptile = ppool.tile([128, 2, TQC], BF16, tag="pt", name="ptile")
                            nc.scalar.activation(
                                out=ptile[:, :, mlo:TQC],
                                in_=ps_gs[hh][:, :, mlo:TQC],
                                func=AF.Exp,
                                scale=SCALE,
                            )
                            ptiles.append(ptile)
                        for hh in range(2):
                            for gi, ik in enumerate(iks):
                                if ik - 4 * jq >= 0:
                                    lo = los[gi]
                                    nc.vector.tensor_mul(
                                        out=ptiles[hh][:, gi, lo : lo + 128],
                                        in0=ptiles[hh][:, gi, lo : lo + 128],
                                        in1=mk_sb[:, 0:128],
                                    )
                        for hh in range(2):
                            for gi, ik in enumerate(iks):
                                lo = los[gi]
                                nc.tensor.matmul(
                                    pys[hh][:, lo:TQC],
                                    lhsT=vhat[:, ik, :],
                                    rhs=ptiles[hh][:, gi, lo:TQC],
                                    start=(ik == 0),
                                    stop=(ik == nik - 1),
                                )
                    boundary(jq)
                    for hh in range(2):
                        # eager copy frees the PSUM accumulator; normalize runs
                        # off the critical path
                        ybuf = small.tile([65, TQC], F32, tag="ybuf", name="ybuf")
                        nc.vector.tensor_copy(out=ybuf[:], in_=pys[hh][:])
                        srow = small.tile([1, TQC], F32, tag="srow", name="srow")
                        nc.vector.tensor_copy(out=srow[:], in_=pys[hh][64:65, :])
                        rinv = small.tile([1, TQC], F32, tag="rinv", name="rinv")
                        nc.vector.reciprocal_approx_fast(out=rinv[:], in_=srow[:])
                        rb = small.tile([64, TQC], F32, tag="rb", name="rb")
                        nc.gpsimd.partition_broadcast(rb[:], rinv[:])
                        nc.vector.tensor_mul(
                            out=yn[hp][hh * 64 : hh * 64 + 64, jq * TQC : (jq + 1) * TQC],
                            in0=ybuf[0:64, :],
                            in1=rb[:],
                        )

            # ---- pass 0 (heads 0,1) with stage-A Q1 interleaved ----
            with (
                tc.tile_pool(name="psS0", bufs=1, space="PSUM") as psS0,
                tc.tile_pool(name="psY0", bufs=2, space="PSUM") as psY0,
                tc.tile_pool(name="psA2", bufs=2, space="PSUM") as psA2,
            ):

                def a2_gen():
                    for jq2 in range(NJQ):
                        pa = psA2.tile([128, TQC], F32, tag="pa2", name="pa2")
                        for cc in range(NCC):
                            nc.tensor.matmul(
                                pa[:],
                                lhsT=W_sb[:, cc, 128:256],
                                rhs=xts[cc][:, jq2 * TQC : (jq2 + 1) * TQC],
                                start=(cc == 0),
                                stop=(cc == NCC - 1),
                            )
                            yield
                        nc.vector.tensor_copy(
                            out=qkv_sb[1][:, jq2 * TQC : (jq2 + 1) * TQC], in_=pa[:]
                        )
                    emit_rope(1)
                    yield

                gen = a2_gen()

                def filler0():
                    next(gen, None)

                def filler0x2():
                    filler0()
                    filler0()

                # head start for the PE while rope-q0 finishes on DVE
                for _ in range(4):
                    filler0()
                attention_pass(
                    0, psS0, psY0, filler0x2, lambda jq: None, jq_order=(3, 2, 1, 0)
                )
                for _ in gen:
                    pass

            # ---- pass 1 (heads 2,3) with projection interleaved ----
            with (
                tc.tile_pool(name="psS1", bufs=1, space="PSUM") as psS1,
                tc.tile_pool(name="psY1", bufs=2, space="PSUM") as psY1,
                tc.tile_pool(name="psP", bufs=1, space="PSUM") as psP,
            ):

                def emit_proj(pjq):
                    for tt in range(4 * pjq, 4 * pjq + 4):
                        outsb = outp.tile([128, C], F32, tag="osb", name="osb")
                        pps = [psP.tile([128, 512], F32, tag=f"pp{n}", name=f"pp{n}") for n in range(2)]
                        for kk in range(2):
                            for ncol in range(2):
                                nc.tensor.matmul(
                                    pps[ncol][:],
                                    lhsT=yn[kk][:, tt * 128 : (tt + 1) * 128],
                                    rhs=wp_sb[:, kk, ncol * 512 : (ncol + 1) * 512],
                                    start=(kk == 0),
                                    stop=(kk == 1),
                                )
                        for ncol in range(2):
                            nc.vector.tensor_copy(
                                out=outsb[:, ncol * 512 : (ncol + 1) * 512], in_=pps[ncol][:]
                            )
                        nc.sync.dma_start(
                            out=out_d[tt * 128 : (tt + 1) * 128, :], in_=outsb[:]
                        )

                def boundary1(jq):
                    if jq > 0:
                        emit_proj(jq - 1)

                attention_pass(1, psS1, psY1, lambda: None, boundary1)
                emit_proj(NJQ - 1)

    nc.compile()
    return nc


def _host_tables():
    # RoPE tables in interleaved-pair device layout (row j'=2i <-> orig j=i,
    # j'=2i+1 <-> orig j=i+32); sign of the shuffled sin term folded in.
    inv = 1.0 / (10000.0 ** (np.arange(0, D, 2, dtype=np.float64) / D))  # (32,)
    t = np.arange(T, dtype=np.float64)
    fr = np.outer(t, inv)  # (T, 32)
    cos_h = np.cos(fr).T.astype(np.float32)  # (32, T)
    sin_h = np.sin(fr).T.astype(np.float32)
    cosI = np.empty((D, T), np.float32)
    sinI = np.empty((D, T), np.float32)
    cosI[0::2] = cos_h
    cosI[1::2] = cos_h
    sinI[0::2] = -sin_h
    sinI[1::2] = sin_h
    cos_q = np.tile(cosI, (2, 1))
    sin_q = np.tile(sinI, (2, 1))
    # tri mask for the diagonal 128-block: allowed iff tkl <= tql
    tkl = np.arange(128)[:, None]
    tql = np.arange(128)[None, :]
    mask = (tkl <= tql).astype(np.float32).astype(ml_dtypes.bfloat16)
    identb = np.tile(np.eye(64, dtype=np.float32), (2, 2))
    return cos_q, sin_q, mask, identb


def make_in_maps(x, wq, wk, wv, wproj):
    cos_q, sin_q, mask, identb = _host_tables()
    # interleave permutation within each head's 64 cols: perm[2i]=i, perm[2i+1]=i+32
    perm = np.empty(D, np.int64)
    perm[0::2] = np.arange(32)
    perm[1::2] = np.arange(32) + 32
    in_maps = []
    for c in range(8):
        b, h = c // 4, c % 4
        xT = np.ascontiguousarray(x[b].T).astype(ml_dtypes.bfloat16)  # (C, T)
        wq_h = wq[:, h * 256 : (h + 1) * 256].reshape(C, G, D)[:, :, perm].reshape(C, 256)
        wk_h = wk[:, h * 64 : (h + 1) * 64][:, perm]
        wv_h = wv[:, h * 64 : (h + 1) * 64]
        w_all = np.concatenate([wq_h, wk_h, wv_h], axis=1).astype(ml_dtypes.bfloat16)
        wp_h = wproj[h * 256 : (h + 1) * 256, :].astype(ml_dtypes.bfloat16)
        in_maps.append(
            {
                "xT": xT,
                "w_all": w_all,
                "wp": wp_h,
                "cos_q": cos_q,
                "sin_q": sin_q,
                "masks": mask,
                "identb": identb,
            }
        )
    return in_maps


def kernel(x, wq, wk, wv, wproj):
    x = np.asarray(x, dtype=np.float32)
    wq = np.asarray(wq, dtype=np.float32)
    wk = np.asarray(wk, dtype=np.float32)
    wv = np.asarray(wv, dtype=np.float32)
    wproj = np.asarray(wproj, dtype=np.float32)
    B = x.shape[0]

    if "nc" not in _PROG:
        _PROG["nc"] = _build_program()
    nc = _PROG["nc"]

    in_maps = make_in_maps(x, wq, wk, wv, wproj)

    res = run_bass_kernel_spmd(nc, in_maps, list(range(8)))
    out = np.zeros((B, T, C), np.float32)
    for c in range(8):
        out[c // 4] += res.results[c]["out_p"]
    return out
